# revision 27
# baseline (speedup 1.0000x reference)
"""Trainium2 Bass kernel: AdditiveAttention scoring head via separable
Fourier-feature expansion.

Reference computes out[b,i-1,j] = sum_a Wout[a]*tanh(x1[b,j,a] + x2[b,i,a])
+ bout with x1 = x@W1.T + b1, x2 = x@W2.T + b2 (B=32, N=128, D=A=512).

Direct evaluation needs B*N*N*A = 268M tanh elements -> ACT-bound (~250us).
Instead approximate tanh(s) ~ sum_k b_k sin(w_k s) (K=10 freqs, two octave
lines {base*2^m}), so tanh(u+v) becomes sum_k b_k [sin_k(u)cos_k(v) +
cos_k(u)sin_k(v)] -- a rank-2K separable form. The NxN cross product then
collapses into TensorEngine matmuls over a (K,A) contraction and the
elementwise work drops to ~262K-element feature streams per core:

  ACT: base sin/cos per octave line (args |w0*y| <= pi/2, in Sin's valid
       range) and Square for sin^2 (both live in the trig_and_small table).
  DVE: frequency doubling s2=s*c (bf16 2x), cos via ts-dual 1-2*sigma^2,
       whose scalar slots also absorb Wout[a]*b_k weighting + cascade scale
       corrections; y staging PSUM->SBUF with bias add.
  PE : x1/x2 input matmuls + 320 accumulating [128p,127m,128n] matmuls
       contracting the feature dim into psum[b][i,j].

Sharding: data-parallel over batch across 8 cores (4 batches/core), weights
replicated, no collectives. Coefficients b_k fit offline (Gaussian-weighted
LS, ridge 1e-6); e2e rel err ~3.8e-3 (tolerance 2e-2).
"""
import sys
import numpy as np

if "/opt/trn_rl_repo" not in sys.path:
    sys.path.insert(0, "/opt/trn_rl_repo")

B, N, D, A = 32, 128, 512, 512
NCORES = 8
BPC = B // NCORES        # batches per core
TOK = BPC * N            # tokens per core (b,n flattened) = 512
KC = D // 128            # d contraction chunks
MC = A // 128            # a chunks
PI = float(np.pi)

# ---- offline Fourier fit of tanh: two octave lines ----
STRUCT = [(0.2038, 5), (0.28, 4)]   # (base freq, levels); freqs base*2^m
RIDGE = 1e-5


def _fit_coeffs():
    sg = np.linspace(-11, 11, 4001)
    w = np.exp(-sg ** 2 / 4.0) + 1e-5
    t = np.tanh(sg)
    freqs = np.concatenate([[bb * 2 ** m for m in range(L)] for bb, L in STRUCT])
    X = np.sin(np.outer(sg, freqs))
    G = X.T @ (X * w[:, None])
    r = X.T @ (t * w)
    bk = np.linalg.solve(G + RIDGE * np.eye(len(freqs)), r)
    return bk.astype(np.float64)


_BK = _fit_coeffs()

_CACHE = {}


def _build_nc():
    import concourse.bass as bass
    import concourse.bacc as bacc
    import concourse.mybir as mybir
    from concourse import tile

    f32 = mybir.dt.float32
    bf16 = mybir.dt.bfloat16
    AF = mybir.ActivationFunctionType
    ALU = mybir.AluOpType

    nc = bacc.Bacc(None, target_bir_lowering=False)

    # DRAM params. xT: [d', k*TOK + t]; wpk: [d', ((side*MC+c)*KC+k)*128 + j]
    big0 = nc.declare_dram_parameter("big0", [128, KC * TOK + MC * KC * 128], bf16, isOutput=False)
    big1 = nc.declare_dram_parameter("big1", [128, MC * KC * 128], bf16, isOutput=False)
    NWC = sum((4 if m == 0 else 8) for bb, L in STRUCT for m in range(L))
    wcoef = nc.declare_dram_parameter("wcoef", [128, NWC], f32, isOutput=False)
    # duplicated-pair bf16 w*coef columns for inner cw TT broadcast:
    # per (line, m<=L-2, chunk): value w_a*coef duplicated in adjacent cols
    NWD = sum(1 for bb, L in STRUCT for m in range(L - 1)) * MC * 2
    wdup = nc.declare_dram_parameter("wdup", [128, NWD], bf16, isOutput=False)
    boutp = nc.declare_dram_parameter("bout", [128, 1], f32, isOutput=False)
    brow = nc.declare_dram_parameter("brow", [1, 2 * A], bf16, isOutput=False)
    out = nc.declare_dram_parameter("out", [N - 1, BPC * N], f32, isOutput=True)

    with tile.TileContext(nc) as tc:
        with (
            tc.tile_pool(name="const", bufs=1) as cpool,
            tc.tile_pool(name="xw", bufs=1) as xwpool,
            tc.tile_pool(name="y", bufs=1) as ypool,
            tc.tile_pool(name="feat", bufs=1) as fpool,
            tc.tile_pool(name="stage", bufs=4) as stpool,
        ):
            # ---- input DMAs: one DMA covers x + side-0 weights ----
            H = MC * KC * 128
            b0sb = xwpool.tile([128, KC * TOK + H], bf16, tag="b0sb")
            nc.sync.dma_start(b0sb[:, :], big0[:, :])
            xsb = b0sb
            b1sb = xwpool.tile([128, H], bf16, tag="b1sb")
            nc.gpsimd.dma_start(b1sb[:, :], big1[:, :])
            brw = cpool.tile([1, 2 * A], bf16, tag="brw")
            nc.scalar.dma_start(brw[:, :], brow[:, :])
            wco = cpool.tile([128, NWC], f32, tag="wco")
            nc.scalar.dma_start(wco[:, :], wcoef[:, :])
            NWD = sum(1 for bb, L in STRUCT for m in range(L - 1)) * MC * 2
            wdu = cpool.tile([128, NWD], bf16, tag="wdu")
            nc.scalar.dma_start(wdu[:, :], wdup[:, :])
            boutf = cpool.tile([128, 1], f32, tag="boutf")
            nc.scalar.dma_start(boutf[:, :], boutp[:, :])
            halfpi = cpool.tile([128, 1], f32, tag="halfpi")
            nc.gpsimd.memset(halfpi[:, :], PI / 2)

            def wslice(side, c, k):
                o = (c * KC + k) * 128
                if side == 0:
                    return b0sb[:, KC * TOK + o:KC * TOK + o + 128]
                return b1sb[:, o:o + 128]

            warm = cpool.tile([128, 128], bf16, tag="warm")
            nc.gpsimd.memset(warm[:, :], 0.25)
            with (
                tc.tile_pool(name="psF", bufs=1, space=bass.MemorySpace.PSUM) as psF,
                tc.tile_pool(name="psS", bufs=1, space=bass.MemorySpace.PSUM) as psS,
                tc.tile_pool(name="psX", bufs=2, space=bass.MemorySpace.PSUM) as psX,
                tc.tile_pool(name="psO", bufs=1, space=bass.MemorySpace.PSUM) as psO,
            ):
                fps = psF.tile([128, 128], f32, tag="psF")

                def filler(n, dep=None):
                    rhs = warm[:, :] if dep is None else dep[:, 0:128]
                    for _ in range(n):
                        nc.tensor.matmul(fps[:, :], warm[:, :], rhs,
                                         start=True, stop=True)
                # ---- x matmuls. side0 psums stay resident for direct Sin;
                # side1 staged to SBUF via ACT Identity (+bias) ----
                y1 = ypool.tile([128, MC * TOK], f32, tag="y1", name="y1")
                ones = cpool.tile([1, TOK], bf16, tag="ones")
                nc.gpsimd.memset(ones[:, :], 1.0)
                filler(6)
                ps0_t = psS.tile([128, MC * TOK], f32, tag="psS", name="ps0")
                for c in range(MC):
                    ps = ps0_t[:, c * TOK:(c + 1) * TOK]
                    for k in range(KC):
                        nc.tensor.matmul(ps, wslice(0, c, k),
                                         xsb[:, k * TOK:(k + 1) * TOK],
                                         start=(k == 0), stop=False,
                                         skip_group_check=True)
                    nc.tensor.matmul(ps, brw[0:1, c * 128:(c + 1) * 128],
                                     ones[0:1, :], start=False, stop=True,
                                     skip_group_check=True)
                for c in range(MC):
                    ps = psX.tile([128, TOK], f32, tag="psX", name=f"psx1_{c}")
                    for k in range(KC):
                        nc.tensor.matmul(ps[:, :], wslice(1, c, k),
                                         xsb[:, k * TOK:(k + 1) * TOK],
                                         start=(k == 0), stop=False,
                                         skip_group_check=True)
                    nc.tensor.matmul(ps[:, :],
                                     brw[0:1, A + c * 128:A + (c + 1) * 128],
                                     ones[0:1, :], start=False, stop=True,
                                     skip_group_check=True)
                    nc.vector.tensor_copy(
                        y1[:, c * TOK:(c + 1) * TOK], ps[:, :])

                pout_t = psO.tile([N - 1, BPC * N], f32, tag="psO", name="pout")
                pouts = [pout_t[:, b * N:(b + 1) * N] for b in range(BPC)]
                zro = cpool.tile([128, BPC * N], bf16, tag="zro")
                nc.gpsimd.memset(zro[:, :], 0.0)
                nc.tensor.matmul(pout_t[:, :], warm[:, 0:N - 1], zro[:, :],
                                 start=True, stop=False, skip_group_check=True)
                FW = MC * TOK

                st = {}       # (li, side, m) -> (s_tile, c_tile)
                sq = {}       # (li, side, m) -> sigma^2 tile
                mm_first = True
                dmaq = [nc.sync, nc.scalar, nc.sync, nc.scalar]

                # wcoef column offsets per (line, m)
                wc_off = []
                o = 0
                for li, (bb, L) in enumerate(STRUCT):
                    offs = []
                    for m in range(L):
                        offs.append(o)
                        o += 4 if m == 0 else 8
                    wc_off.append(offs)
                # wdup column offsets per (line, m<=L-2): MC*2 bf16 cols each
                wd_off = []
                o = 0
                for li, (bb, L) in enumerate(STRUCT):
                    offs = []
                    for m in range(L - 1):
                        offs.append(o)
                        o += MC * 2
                    wd_off.append(offs)

                def make_cw_tt(li, m, side, c_tile):
                    # cw = c_m * (w_a*coef) via TT-mult against dup-pair
                    # broadcast AP (keeps DVE 2x mode)
                    cw = fpool.tile([128, FW], bf16, tag=f"cw_{li}_{side}_p{m % 2}",
                                    name=f"cw{li}{side}{m}")
                    wap = wdu[:, :]
                    in1 = bass.AP(wap.tensor, wap.offset + wd_off[li][m],
                                  [[wap.ap[0][0], 128], [2, MC], [0, TOK // 2],
                                   [1, 2]])
                    cap = c_tile[:, :]
                    in0 = bass.AP(cap.tensor, cap.offset,
                                  [[cap.ap[0][0], 128], [TOK, MC], [2, TOK // 2],
                                   [1, 2]])
                    oap = cw[:, :]
                    o0 = bass.AP(oap.tensor, oap.offset,
                                 [[oap.ap[0][0], 128], [TOK, MC], [2, TOK // 2],
                                  [1, 2]])
                    nc.vector.tensor_tensor(o0, in0, in1, ALU.mult)
                    return cw

                def make_cw_leaf(li, m, side, sq_tile):
                    # leaf: cw = sigma^2*(-2 w coef/lam^2) + w coef (dual ts)
                    # as per-chunk tiles so final mms start on chunk 0
                    cws = []
                    for c in range(MC):
                        cwc = fpool.tile([128, TOK], bf16,
                                         tag=f"cwl_{li}_{side}_{c}",
                                         name=f"cwl{li}{side}{m}{c}")
                        col = wc_off[li][m] + 2 * c
                        nc.vector.tensor_scalar(
                            cwc[:, :],
                            sq_tile[:, c * TOK:(c + 1) * TOK],
                            wco[:, col:col + 1], wco[:, col + 1:col + 2],
                            ALU.mult, ALU.add)
                        cws.append(cwc)
                    return cws

                def emit_mms(li, m, cws, last):
                    # A: psum[i,j] += sum_a c2w[a,i]*s1[a,j]
                    # B: psum[i,j] += sum_a s2[a,i]*c1w[a,j]
                    nonlocal mm_first
                    s1_t = st[(li, 0, m)][0]
                    s2_t = st[(li, 1, m)][0]
                    c1w_t, c2w_t = cws

                    def cw_sl(t, c, lo, hi):
                        if isinstance(t, list):
                            return t[c][:, lo:hi]
                        return t[:, c * TOK + lo:c * TOK + hi]
                    if not last:
                        for c in range(MC):
                            for b in range(BPC):
                                t0 = c * TOK + b * N
                                nc.tensor.matmul(
                                    pouts[b],
                                    cw_sl(c2w_t, c, b * N + 1, b * N + N),
                                    s1_t[:, t0:t0 + N],
                                    start=False, stop=False,
                                    skip_group_check=True)
                                nc.tensor.matmul(
                                    pouts[b],
                                    s2_t[:, t0 + 1:t0 + N],
                                    cw_sl(c1w_t, c, b * N, b * N + N),
                                    start=False, stop=False,
                                    skip_group_check=True)
                        mm_first = False
                        return
                    # final pair: batch-major mms, stage into one wide tile,
                    # two big (>=4KB/partition-row) DMAs
                    stg = stpool.tile([N - 1, BPC * N], f32, tag="stg")
                    for b in range(BPC):
                        for c in range(MC):
                            t0 = c * TOK + b * N
                            nc.tensor.matmul(
                                pouts[b],
                                cw_sl(c2w_t, c, b * N + 1, b * N + N),
                                s1_t[:, t0:t0 + N],
                                start=False, stop=False,
                                skip_group_check=True)
                            nc.tensor.matmul(
                                pouts[b],
                                s2_t[:, t0 + 1:t0 + N],
                                cw_sl(c1w_t, c, b * N, b * N + N),
                                start=False,
                                stop=(c == MC - 1),
                                skip_group_check=True)
                        nc.scalar.activation(stg[:, b * N:(b + 1) * N],
                                             pouts[b], AF.Identity,
                                             bias=boutf[0:N - 1, 0:1])
                        if b == 1:
                            nc.sync.dma_start(out[:, 0:2 * N], stg[:, 0:2 * N])
                        elif b == BPC - 1:
                            nc.scalar.dma_start(out[:, 2 * N:], stg[:, 2 * N:])

                # ---- bases: per line, side0 direct from PSUM then side1
                # from y1, so ACT starts as soon as side-0 psums land ----
                for li, (bb, L) in enumerate(STRUCT):
                    for ph in range(2):
                        f0 = fpool.tile([128, FW], bf16,
                                        tag=("s" if ph == 0 else "c") + f"_{li}_0_p0",
                                        name=("s" if ph == 0 else "c") + f"{li}00")
                        nc.scalar.activation(f0[:, :], ps0_t[:, :], AF.Sin,
                                             bias=(0.0 if ph == 0
                                                   else halfpi[:, 0:1]),
                                             scale=float(bb))
                        if ph == 0:
                            s0_0 = f0
                        else:
                            st[(li, 0, 0)] = (s0_0, f0)
                    s0 = fpool.tile([128, FW], bf16, tag=f"s_{li}_1_p0",
                                    name=f"s{li}10")
                    nc.scalar.activation(s0[:, :], y1[:, :], AF.Sin,
                                         bias=0.0, scale=float(bb))
                    c0 = fpool.tile([128, FW], bf16, tag=f"c_{li}_1_p0",
                                    name=f"c{li}10")
                    nc.scalar.activation(c0[:, :], y1[:, :], AF.Sin,
                                         bias=halfpi[:, 0:1], scale=float(bb))
                    st[(li, 1, 0)] = (s0, c0)
                for li, (bb, L) in enumerate(STRUCT):
                    for side in range(2):
                        t = fpool.tile([128, FW], bf16, tag=f"sq_{li}_{side}",
                                       name=f"sq{li}{side}0")
                        nc.scalar.activation(t[:, :], st[(li, side, 0)][0][:, :],
                                             AF.Square)
                        sq[(li, side, 0)] = t
                for li, (bb, L) in enumerate(STRUCT):
                    cws = [make_cw_tt(li, 0, side, st[(li, side, 0)][1])
                           for side in range(2)]
                    emit_mms(li, 0, cws, last=False)
                    filler(2, dep=st[(li, 0, 0)][0])

                # ---- staggered level schedule ----
                steps = []
                Lmax = max(L for _, L in STRUCT)
                for m in range(1, Lmax):
                    for li, (bb, L) in enumerate(STRUCT):
                        if m < L:
                            steps.append((li, m))
                for si, (li, m) in enumerate(steps):
                    bb, L = STRUCT[li]
                    lam_prev = 0.5 ** (m - 1)
                    is_last = si == len(steps) - 1
                    cms = [None, None]
                    if m <= L - 2:
                        for side in range(2):
                            cm = fpool.tile([128, FW], bf16,
                                            tag=f"c_{li}_{side}_p{m % 2}",
                                            name=f"c{li}{side}{m}")
                            nc.vector.tensor_scalar(
                                cm[:, :], sq[(li, side, m - 1)][:, :],
                                float(-2.0 / lam_prev ** 2), 1.0,
                                ALU.mult, ALU.add)
                            cms[side] = cm

                    def make_cw_for(side):
                        if m <= L - 2:
                            return make_cw_tt(li, m, side, cms[side])
                        return make_cw_leaf(li, m, side, sq[(li, side, m - 1)])

                    def make_s(side):
                        sp, cp = st[(li, side, m - 1)]
                        sm = fpool.tile([128, FW], bf16,
                                        tag=f"s_{li}_{side}_p{m % 2}",
                                        name=f"s{li}{side}{m}")
                        nc.vector.tensor_tensor(sm[:, :], sp[:, :], cp[:, :],
                                                ALU.mult)
                        return sm

                    if m <= L - 2:
                        cw1 = make_cw_for(1)
                        s1m = make_s(0)
                        cw0 = make_cw_for(0)
                        s2m = make_s(1)
                    else:
                        s1m = make_s(0)
                        s2m = make_s(1)
                        filler(6, dep=s1m)
                        cw1 = make_cw_for(1)
                        cw0 = make_cw_for(0)
                    st[(li, 0, m)] = (s1m, cms[0])
                    st[(li, 1, m)] = (s2m, cms[1])
                    if m <= L - 2:
                        for side, smt in ((0, s1m), (1, s2m)):
                            t = fpool.tile([128, FW], bf16, tag=f"sq_{li}_{side}",
                                           name=f"sq{li}{side}{m}")
                            nc.scalar.activation(t[:, :], smt[:, :], AF.Square)
                            sq[(li, side, m)] = t
                    emit_mms(li, m, [cw0, cw1], last=is_last)
                    if not is_last:
                        filler(2, dep=st[(li, 0, m - 1)][0])

    nc.finalize()
    return nc


def _get_nc():
    if "nc" not in _CACHE:
        _CACHE["nc"] = _build_nc()
    return _CACHE["nc"]


def _prep_in_maps(x, W1, b1, W2, b2, Wout, bout):
    import ml_dtypes
    f = np.float32
    bfd = ml_dtypes.bfloat16
    x = np.asarray(x, f)
    W1 = np.asarray(W1, f)
    W2 = np.asarray(W2, f)
    Wo = np.asarray(Wout, f)

    # weights: wpk[d', ((side*MC+c)*KC+k)*128 + j] = Wside[c*128+j, k*128+d']
    def pack_w(W):
        # [MC, 128j, KC, 128d'] -> transpose to [128d', MC, KC, 128j]
        t = W.reshape(MC, 128, KC, 128).transpose(3, 0, 2, 1)
        return np.ascontiguousarray(t.reshape(128, MC * KC * 128)).astype(f)

    w1p = pack_w(W1)
    w2p = pack_w(W2)
    browv = np.concatenate([np.asarray(b1, f), np.asarray(b2, f)]).reshape(1, 2 * A).astype(bfd)

    # wcoef table
    NWC = sum((4 if m == 0 else 8) for bb, L in STRUCT for m in range(L))
    wco = np.zeros((128, NWC), f)
    o = 0
    ki = 0
    for bb, L in STRUCT:
        for m in range(L):
            lam = 0.5 ** m
            coef = _BK[ki + m] / lam
            if m == 0:
                for c in range(MC):
                    wco[:, o + c] = Wo[c * 128:(c + 1) * 128] * coef
                o += 4
            else:
                lam_prev = 0.5 ** (m - 1)
                for c in range(MC):
                    wchunk = Wo[c * 128:(c + 1) * 128]
                    wco[:, o + 2 * c] = wchunk * (-2.0 * coef / lam_prev ** 2)
                    wco[:, o + 2 * c + 1] = wchunk * coef
                o += 8
        ki += L
    bov = np.full((128, 1), np.asarray(bout, f).reshape(()), f)

    # duplicated-pair bf16 table for inner cw TT: per (line, m<=L-2, chunk)
    NWD = sum(1 for bb, L in STRUCT for m in range(L - 1)) * MC * 2
    wdu = np.zeros((128, NWD), f)
    o = 0
    ki = 0
    for bb, L in STRUCT:
        for m in range(L - 1):
            coef = _BK[ki + m] / (0.5 ** m)
            for c in range(MC):
                v = Wo[c * 128:(c + 1) * 128] * coef
                wdu[:, o + 2 * c] = v
                wdu[:, o + 2 * c + 1] = v
            o += MC * 2
        ki += L
    wdu = wdu.astype(bfd)

    in_maps = []
    for ci in range(NCORES):
        xs = x[ci * BPC:(ci + 1) * BPC]          # [BPC, N, D]
        xt = xs.transpose(2, 0, 1).reshape(D, TOK)   # [D, TOK]
        # -> [128, KC*TOK]: xTi[d', k*TOK + t] = xt[k*128+d', t]
        xTi = np.ascontiguousarray(
            xt.reshape(KC, 128, TOK).transpose(1, 0, 2).reshape(128, KC * TOK))
        in_maps.append({
            "big0": np.ascontiguousarray(np.concatenate([xTi.astype(np.float32),
                                                         w1p], axis=1)).astype(bfd),
            "big1": w2p.astype(bfd), "brow": browv,
            "wcoef": wco, "bout": bov, "wdup": wdu,
        })
    return in_maps


def _run(x, W1, b1, W2, b2, Wout, bout, trace=False):
    from concourse.bass_utils import run_bass_kernel_spmd

    nc = _get_nc()
    in_maps = _prep_in_maps(x, W1, b1, W2, b2, Wout, bout)
    res = run_bass_kernel_spmd(nc, in_maps, core_ids=list(range(NCORES)),
                               trace=trace)
    outs = [np.asarray(res.results[ci]["out"]).reshape(N - 1, BPC, N)
            .transpose(1, 0, 2) for ci in range(NCORES)]
    full = np.concatenate(outs, axis=0).astype(np.float32)
    return full, res


def kernel(x, W1, b1, W2, b2, Wout, bout):
    full, _ = _run(x, W1, b1, W2, b2, Wout, bout, trace=False)
    return full


# revision 28
# speedup vs baseline: 1.0188x; 1.0188x over previous
"""Trainium2 Bass kernel: AdditiveAttention scoring head via separable
Fourier-feature expansion.

Reference computes out[b,i-1,j] = sum_a Wout[a]*tanh(x1[b,j,a] + x2[b,i,a])
+ bout with x1 = x@W1.T + b1, x2 = x@W2.T + b2 (B=32, N=128, D=A=512).

Direct evaluation needs B*N*N*A = 268M tanh elements -> ACT-bound (~250us).
Instead approximate tanh(s) ~ sum_k b_k sin(w_k s) (K=10 freqs, two octave
lines {base*2^m}), so tanh(u+v) becomes sum_k b_k [sin_k(u)cos_k(v) +
cos_k(u)sin_k(v)] -- a rank-2K separable form. The NxN cross product then
collapses into TensorEngine matmuls over a (K,A) contraction and the
elementwise work drops to ~262K-element feature streams per core:

  ACT: base sin/cos per octave line (args |w0*y| <= pi/2, in Sin's valid
       range) and Square for sin^2 (both live in the trig_and_small table).
  DVE: frequency doubling s2=s*c (bf16 2x), cos via ts-dual 1-2*sigma^2,
       whose scalar slots also absorb Wout[a]*b_k weighting + cascade scale
       corrections; y staging PSUM->SBUF with bias add.
  PE : x1/x2 input matmuls + 320 accumulating [128p,127m,128n] matmuls
       contracting the feature dim into psum[b][i,j].

Sharding: data-parallel over batch across 8 cores (4 batches/core), weights
replicated, no collectives. Coefficients b_k fit offline (Gaussian-weighted
LS, ridge 1e-6); e2e rel err ~3.8e-3 (tolerance 2e-2).
"""
import sys
import numpy as np

if "/opt/trn_rl_repo" not in sys.path:
    sys.path.insert(0, "/opt/trn_rl_repo")

B, N, D, A = 32, 128, 512, 512
NCORES = 8
BPC = B // NCORES        # batches per core
TOK = BPC * N            # tokens per core (b,n flattened) = 512
KC = D // 128            # d contraction chunks
MC = A // 128            # a chunks
PI = float(np.pi)

# ---- offline Fourier fit of tanh: two octave lines ----
STRUCT = [(0.2038, 5), (0.28, 4)]   # (base freq, levels); freqs base*2^m
RIDGE = 1e-5


def _fit_coeffs():
    sg = np.linspace(-11, 11, 4001)
    w = np.exp(-sg ** 2 / 4.0) + 1e-5
    t = np.tanh(sg)
    freqs = np.concatenate([[bb * 2 ** m for m in range(L)] for bb, L in STRUCT])
    X = np.sin(np.outer(sg, freqs))
    G = X.T @ (X * w[:, None])
    r = X.T @ (t * w)
    bk = np.linalg.solve(G + RIDGE * np.eye(len(freqs)), r)
    return bk.astype(np.float64)


_BK = _fit_coeffs()

_CACHE = {}


def _build_nc():
    import concourse.bass as bass
    import concourse.bacc as bacc
    import concourse.mybir as mybir
    from concourse import tile

    f32 = mybir.dt.float32
    bf16 = mybir.dt.bfloat16
    AF = mybir.ActivationFunctionType
    ALU = mybir.AluOpType

    nc = bacc.Bacc(None, target_bir_lowering=False)

    # DRAM params. xT: [d', k*TOK + t]; wpk: [d', ((side*MC+c)*KC+k)*128 + j]
    big0 = nc.declare_dram_parameter("big0", [128, KC * TOK + MC * KC * 128], bf16, isOutput=False)
    big1 = nc.declare_dram_parameter("big1", [128, MC * KC * 128], bf16, isOutput=False)
    NWC = sum((4 if m == 0 else 8) for bb, L in STRUCT for m in range(L))
    wcoef = nc.declare_dram_parameter("wcoef", [128, NWC], f32, isOutput=False)
    # duplicated-pair bf16 w*coef columns for inner cw TT broadcast:
    # per (line, m<=L-2, chunk): value w_a*coef duplicated in adjacent cols
    NWD = sum(1 for bb, L in STRUCT for m in range(L - 1)) * MC * 2
    wdup = nc.declare_dram_parameter("wdup", [128, NWD], bf16, isOutput=False)
    boutp = nc.declare_dram_parameter("bout", [128, 1], f32, isOutput=False)
    brow = nc.declare_dram_parameter("brow", [1, 2 * A], bf16, isOutput=False)
    out = nc.declare_dram_parameter("out", [N - 1, BPC * N], f32, isOutput=True)

    with tile.TileContext(nc) as tc:
        with (
            tc.tile_pool(name="const", bufs=1) as cpool,
            tc.tile_pool(name="xw", bufs=1) as xwpool,
            tc.tile_pool(name="y", bufs=1) as ypool,
            tc.tile_pool(name="feat", bufs=1) as fpool,
            tc.tile_pool(name="stage", bufs=4) as stpool,
        ):
            # ---- input DMAs: one DMA covers x + side-0 weights ----
            H = MC * KC * 128
            b0sb = xwpool.tile([128, KC * TOK + H], bf16, tag="b0sb")
            nc.sync.dma_start(b0sb[:, :], big0[:, :])
            xsb = b0sb
            b1sb = xwpool.tile([128, H], bf16, tag="b1sb")
            nc.gpsimd.dma_start(b1sb[:, :], big1[:, :])
            brw = cpool.tile([1, 2 * A], bf16, tag="brw")
            nc.scalar.dma_start(brw[:, :], brow[:, :])
            wco = cpool.tile([128, NWC], f32, tag="wco")
            nc.scalar.dma_start(wco[:, :], wcoef[:, :])
            NWD = sum(1 for bb, L in STRUCT for m in range(L - 1)) * MC * 2
            wdu = cpool.tile([128, NWD], bf16, tag="wdu")
            nc.scalar.dma_start(wdu[:, :], wdup[:, :])
            boutf = cpool.tile([128, 1], f32, tag="boutf")
            nc.scalar.dma_start(boutf[:, :], boutp[:, :])
            halfpi = cpool.tile([128, 1], f32, tag="halfpi")
            nc.gpsimd.memset(halfpi[:, :], PI / 2)

            def wslice(side, c, k):
                o = (c * KC + k) * 128
                if side == 0:
                    return b0sb[:, KC * TOK + o:KC * TOK + o + 128]
                return b1sb[:, o:o + 128]

            warm = cpool.tile([128, 128], bf16, tag="warm")
            nc.gpsimd.memset(warm[:, :], 0.25)
            with (
                tc.tile_pool(name="psF", bufs=1, space=bass.MemorySpace.PSUM) as psF,
                tc.tile_pool(name="psS", bufs=4, space=bass.MemorySpace.PSUM) as psS,
                tc.tile_pool(name="psX", bufs=2, space=bass.MemorySpace.PSUM) as psX,
                tc.tile_pool(name="psO", bufs=1, space=bass.MemorySpace.PSUM) as psO,
            ):
                fps = psF.tile([128, 128], f32, tag="psF")

                def filler(n, dep=None):
                    rhs = warm[:, :] if dep is None else dep[:, 0:128]
                    for _ in range(n):
                        nc.tensor.matmul(fps[:, :], warm[:, :], rhs,
                                         start=True, stop=True)
                # ---- x matmuls. side0 psums stay resident for direct Sin;
                # side1 staged to SBUF via ACT Identity (+bias) ----
                y1 = ypool.tile([128, MC * TOK], f32, tag="y1", name="y1")
                ones = cpool.tile([1, TOK], bf16, tag="ones")
                nc.gpsimd.memset(ones[:, :], 1.0)
                filler(6)
                ps0 = []
                for c in range(MC):
                    ps = psS.tile([128, TOK], f32, tag="psS", name=f"ps0_{c}")
                    for k in range(KC):
                        nc.tensor.matmul(ps[:, :], wslice(0, c, k),
                                         xsb[:, k * TOK:(k + 1) * TOK],
                                         start=(k == 0), stop=False,
                                         skip_group_check=True)
                    nc.tensor.matmul(ps[:, :], brw[0:1, c * 128:(c + 1) * 128],
                                     ones[0:1, :], start=False, stop=True,
                                     skip_group_check=True)
                    ps0.append(ps)
                for c in range(MC):
                    ps = psX.tile([128, TOK], f32, tag="psX", name=f"psx1_{c}")
                    for k in range(KC):
                        nc.tensor.matmul(ps[:, :], wslice(1, c, k),
                                         xsb[:, k * TOK:(k + 1) * TOK],
                                         start=(k == 0), stop=False,
                                         skip_group_check=True)
                    nc.tensor.matmul(ps[:, :],
                                     brw[0:1, A + c * 128:A + (c + 1) * 128],
                                     ones[0:1, :], start=False, stop=True,
                                     skip_group_check=True)
                    nc.vector.tensor_copy(
                        y1[:, c * TOK:(c + 1) * TOK], ps[:, :])

                pout_t = psO.tile([N - 1, BPC * N], f32, tag="psO", name="pout")
                pouts = [pout_t[:, b * N:(b + 1) * N] for b in range(BPC)]
                zro = cpool.tile([128, BPC * N], bf16, tag="zro")
                nc.gpsimd.memset(zro[:, :], 0.0)
                nc.tensor.matmul(pout_t[:, :], warm[:, 0:N - 1], zro[:, :],
                                 start=True, stop=False, skip_group_check=True)
                FW = MC * TOK

                st = {}       # (li, side, m) -> (s_tile, c_tile)
                sq = {}       # (li, side, m) -> sigma^2 tile
                mm_first = True
                dmaq = [nc.sync, nc.scalar, nc.sync, nc.scalar]

                # wcoef column offsets per (line, m)
                wc_off = []
                o = 0
                for li, (bb, L) in enumerate(STRUCT):
                    offs = []
                    for m in range(L):
                        offs.append(o)
                        o += 4 if m == 0 else 8
                    wc_off.append(offs)
                # wdup column offsets per (line, m<=L-2): MC*2 bf16 cols each
                wd_off = []
                o = 0
                for li, (bb, L) in enumerate(STRUCT):
                    offs = []
                    for m in range(L - 1):
                        offs.append(o)
                        o += MC * 2
                    wd_off.append(offs)

                def make_cw_tt(li, m, side, c_tile):
                    # cw = c_m * (w_a*coef) via TT-mult against dup-pair
                    # broadcast AP (keeps DVE 2x mode)
                    cw = fpool.tile([128, FW], bf16, tag=f"cw_{li}_{side}_p{m % 2}",
                                    name=f"cw{li}{side}{m}")
                    wap = wdu[:, :]
                    in1 = bass.AP(wap.tensor, wap.offset + wd_off[li][m],
                                  [[wap.ap[0][0], 128], [2, MC], [0, TOK // 2],
                                   [1, 2]])
                    cap = c_tile[:, :]
                    in0 = bass.AP(cap.tensor, cap.offset,
                                  [[cap.ap[0][0], 128], [TOK, MC], [2, TOK // 2],
                                   [1, 2]])
                    oap = cw[:, :]
                    o0 = bass.AP(oap.tensor, oap.offset,
                                 [[oap.ap[0][0], 128], [TOK, MC], [2, TOK // 2],
                                  [1, 2]])
                    nc.vector.tensor_tensor(o0, in0, in1, ALU.mult)
                    return cw

                def make_cw_leaf(li, m, side, sq_tile):
                    # leaf: cw = sigma^2*(-2 w coef/lam^2) + w coef (dual ts)
                    # as per-chunk tiles so final mms start on chunk 0
                    cws = []
                    for c in range(MC):
                        cwc = fpool.tile([128, TOK], bf16,
                                         tag=f"cwl_{li}_{side}_{c}",
                                         name=f"cwl{li}{side}{m}{c}")
                        col = wc_off[li][m] + 2 * c
                        nc.vector.tensor_scalar(
                            cwc[:, :],
                            sq_tile[:, c * TOK:(c + 1) * TOK],
                            wco[:, col:col + 1], wco[:, col + 1:col + 2],
                            ALU.mult, ALU.add)
                        cws.append(cwc)
                    return cws

                def emit_mms(li, m, cws, last):
                    # A: psum[i,j] += sum_a c2w[a,i]*s1[a,j]
                    # B: psum[i,j] += sum_a s2[a,i]*c1w[a,j]
                    nonlocal mm_first
                    s1_t = st[(li, 0, m)][0]
                    s2_t = st[(li, 1, m)][0]
                    c1w_t, c2w_t = cws

                    def cw_sl(t, c, lo, hi):
                        if isinstance(t, list):
                            return t[c][:, lo:hi]
                        return t[:, c * TOK + lo:c * TOK + hi]
                    if not last:
                        for c in range(MC):
                            for b in range(BPC):
                                t0 = c * TOK + b * N
                                nc.tensor.matmul(
                                    pouts[b],
                                    cw_sl(c2w_t, c, b * N + 1, b * N + N),
                                    s1_t[:, t0:t0 + N],
                                    start=False, stop=False,
                                    skip_group_check=True)
                                nc.tensor.matmul(
                                    pouts[b],
                                    s2_t[:, t0 + 1:t0 + N],
                                    cw_sl(c1w_t, c, b * N, b * N + N),
                                    start=False, stop=False,
                                    skip_group_check=True)
                        mm_first = False
                        return
                    # final pair: batch-major mms, stage into one wide tile,
                    # two big (>=4KB/partition-row) DMAs
                    stg = stpool.tile([N - 1, BPC * N], f32, tag="stg")
                    for b in range(BPC):
                        for c in range(MC):
                            t0 = c * TOK + b * N
                            nc.tensor.matmul(
                                pouts[b],
                                cw_sl(c2w_t, c, b * N + 1, b * N + N),
                                s1_t[:, t0:t0 + N],
                                start=False, stop=False,
                                skip_group_check=True)
                            nc.tensor.matmul(
                                pouts[b],
                                s2_t[:, t0 + 1:t0 + N],
                                cw_sl(c1w_t, c, b * N, b * N + N),
                                start=False,
                                stop=(c == MC - 1),
                                skip_group_check=True)
                        nc.scalar.activation(stg[:, b * N:(b + 1) * N],
                                             pouts[b], AF.Identity,
                                             bias=boutf[0:N - 1, 0:1])
                        if b == 1:
                            nc.sync.dma_start(out[:, 0:2 * N], stg[:, 0:2 * N])
                        elif b == BPC - 1:
                            nc.scalar.dma_start(out[:, 2 * N:], stg[:, 2 * N:])

                # ---- bases: per line, side0 direct from PSUM then side1
                # from y1, so ACT starts as soon as side-0 psums land ----
                for li, (bb, L) in enumerate(STRUCT):
                    for ph in range(2):
                        f0 = fpool.tile([128, FW], bf16,
                                        tag=("s" if ph == 0 else "c") + f"_{li}_0_p0",
                                        name=("s" if ph == 0 else "c") + f"{li}00")
                        for c in range(MC):
                            nc.scalar.activation(f0[:, c * TOK:(c + 1) * TOK],
                                                 ps0[c][:, :], AF.Sin,
                                                 bias=(0.0 if ph == 0
                                                       else halfpi[:, 0:1]),
                                                 scale=float(bb))
                        if ph == 0:
                            s0_0 = f0
                        else:
                            st[(li, 0, 0)] = (s0_0, f0)
                    s0 = fpool.tile([128, FW], bf16, tag=f"s_{li}_1_p0",
                                    name=f"s{li}10")
                    nc.scalar.activation(s0[:, :], y1[:, :], AF.Sin,
                                         bias=0.0, scale=float(bb))
                    c0 = fpool.tile([128, FW], bf16, tag=f"c_{li}_1_p0",
                                    name=f"c{li}10")
                    nc.scalar.activation(c0[:, :], y1[:, :], AF.Sin,
                                         bias=halfpi[:, 0:1], scale=float(bb))
                    st[(li, 1, 0)] = (s0, c0)
                for li, (bb, L) in enumerate(STRUCT):
                    for side in range(2):
                        t = fpool.tile([128, FW], bf16, tag=f"sq_{li}_{side}",
                                       name=f"sq{li}{side}0")
                        nc.scalar.activation(t[:, :], st[(li, side, 0)][0][:, :],
                                             AF.Square)
                        sq[(li, side, 0)] = t
                for li, (bb, L) in enumerate(STRUCT):
                    cws = [make_cw_tt(li, 0, side, st[(li, side, 0)][1])
                           for side in range(2)]
                    emit_mms(li, 0, cws, last=False)
                    filler(2, dep=st[(li, 0, 0)][0])

                # ---- staggered level schedule ----
                steps = []
                Lmax = max(L for _, L in STRUCT)
                for m in range(1, Lmax):
                    for li, (bb, L) in enumerate(STRUCT):
                        if m < L:
                            steps.append((li, m))
                for si, (li, m) in enumerate(steps):
                    bb, L = STRUCT[li]
                    lam_prev = 0.5 ** (m - 1)
                    is_last = si == len(steps) - 1
                    cms = [None, None]
                    if m <= L - 2:
                        for side in range(2):
                            cm = fpool.tile([128, FW], bf16,
                                            tag=f"c_{li}_{side}_p{m % 2}",
                                            name=f"c{li}{side}{m}")
                            nc.vector.tensor_scalar(
                                cm[:, :], sq[(li, side, m - 1)][:, :],
                                float(-2.0 / lam_prev ** 2), 1.0,
                                ALU.mult, ALU.add)
                            cms[side] = cm

                    def make_cw_for(side):
                        if m <= L - 2:
                            return make_cw_tt(li, m, side, cms[side])
                        return make_cw_leaf(li, m, side, sq[(li, side, m - 1)])

                    def make_s(side):
                        sp, cp = st[(li, side, m - 1)]
                        sm = fpool.tile([128, FW], bf16,
                                        tag=f"s_{li}_{side}_p{m % 2}",
                                        name=f"s{li}{side}{m}")
                        nc.vector.tensor_tensor(sm[:, :], sp[:, :], cp[:, :],
                                                ALU.mult)
                        return sm

                    if m <= L - 2:
                        cw1 = make_cw_for(1)
                        s1m = make_s(0)
                        cw0 = make_cw_for(0)
                        s2m = make_s(1)
                    else:
                        s1m = make_s(0)
                        s2m = make_s(1)
                        filler(6, dep=s1m)
                        cw1 = make_cw_for(1)
                        cw0 = make_cw_for(0)
                    st[(li, 0, m)] = (s1m, cms[0])
                    st[(li, 1, m)] = (s2m, cms[1])
                    if m <= L - 2:
                        for side, smt in ((0, s1m), (1, s2m)):
                            t = fpool.tile([128, FW], bf16, tag=f"sq_{li}_{side}",
                                           name=f"sq{li}{side}{m}")
                            nc.scalar.activation(t[:, :], smt[:, :], AF.Square)
                            sq[(li, side, m)] = t
                    emit_mms(li, m, [cw0, cw1], last=is_last)
                    if not is_last:
                        filler(2, dep=st[(li, 0, m - 1)][0])

    nc.finalize()
    return nc


def _get_nc():
    if "nc" not in _CACHE:
        _CACHE["nc"] = _build_nc()
    return _CACHE["nc"]


def _prep_in_maps(x, W1, b1, W2, b2, Wout, bout):
    import ml_dtypes
    f = np.float32
    bfd = ml_dtypes.bfloat16
    x = np.asarray(x, f)
    W1 = np.asarray(W1, f)
    W2 = np.asarray(W2, f)
    Wo = np.asarray(Wout, f)

    # weights: wpk[d', ((side*MC+c)*KC+k)*128 + j] = Wside[c*128+j, k*128+d']
    def pack_w(W):
        # [MC, 128j, KC, 128d'] -> transpose to [128d', MC, KC, 128j]
        t = W.reshape(MC, 128, KC, 128).transpose(3, 0, 2, 1)
        return np.ascontiguousarray(t.reshape(128, MC * KC * 128)).astype(f)

    w1p = pack_w(W1)
    w2p = pack_w(W2)
    browv = np.concatenate([np.asarray(b1, f), np.asarray(b2, f)]).reshape(1, 2 * A).astype(bfd)

    # wcoef table
    NWC = sum((4 if m == 0 else 8) for bb, L in STRUCT for m in range(L))
    wco = np.zeros((128, NWC), f)
    o = 0
    ki = 0
    for bb, L in STRUCT:
        for m in range(L):
            lam = 0.5 ** m
            coef = _BK[ki + m] / lam
            if m == 0:
                for c in range(MC):
                    wco[:, o + c] = Wo[c * 128:(c + 1) * 128] * coef
                o += 4
            else:
                lam_prev = 0.5 ** (m - 1)
                for c in range(MC):
                    wchunk = Wo[c * 128:(c + 1) * 128]
                    wco[:, o + 2 * c] = wchunk * (-2.0 * coef / lam_prev ** 2)
                    wco[:, o + 2 * c + 1] = wchunk * coef
                o += 8
        ki += L
    bov = np.full((128, 1), np.asarray(bout, f).reshape(()), f)

    # duplicated-pair bf16 table for inner cw TT: per (line, m<=L-2, chunk)
    NWD = sum(1 for bb, L in STRUCT for m in range(L - 1)) * MC * 2
    wdu = np.zeros((128, NWD), f)
    o = 0
    ki = 0
    for bb, L in STRUCT:
        for m in range(L - 1):
            coef = _BK[ki + m] / (0.5 ** m)
            for c in range(MC):
                v = Wo[c * 128:(c + 1) * 128] * coef
                wdu[:, o + 2 * c] = v
                wdu[:, o + 2 * c + 1] = v
            o += MC * 2
        ki += L
    wdu = wdu.astype(bfd)

    in_maps = []
    for ci in range(NCORES):
        xs = x[ci * BPC:(ci + 1) * BPC]          # [BPC, N, D]
        xt = xs.transpose(2, 0, 1).reshape(D, TOK)   # [D, TOK]
        # -> [128, KC*TOK]: xTi[d', k*TOK + t] = xt[k*128+d', t]
        xTi = np.ascontiguousarray(
            xt.reshape(KC, 128, TOK).transpose(1, 0, 2).reshape(128, KC * TOK))
        in_maps.append({
            "big0": np.ascontiguousarray(np.concatenate([xTi.astype(np.float32),
                                                         w1p], axis=1)).astype(bfd),
            "big1": w2p.astype(bfd), "brow": browv,
            "wcoef": wco, "bout": bov, "wdup": wdu,
        })
    return in_maps


def _run(x, W1, b1, W2, b2, Wout, bout, trace=False):
    from concourse.bass_utils import run_bass_kernel_spmd

    nc = _get_nc()
    in_maps = _prep_in_maps(x, W1, b1, W2, b2, Wout, bout)
    res = run_bass_kernel_spmd(nc, in_maps, core_ids=list(range(NCORES)),
                               trace=trace)
    outs = [np.asarray(res.results[ci]["out"]).reshape(N - 1, BPC, N)
            .transpose(1, 0, 2) for ci in range(NCORES)]
    full = np.concatenate(outs, axis=0).astype(np.float32)
    return full, res


def kernel(x, W1, b1, W2, b2, Wout, bout):
    full, _ = _run(x, W1, b1, W2, b2, Wout, bout, trace=False)
    return full


# revision 29
# speedup vs baseline: 1.0362x; 1.0171x over previous
"""Trainium2 Bass kernel: AdditiveAttention scoring head via separable
Fourier-feature expansion.

Reference computes out[b,i-1,j] = sum_a Wout[a]*tanh(x1[b,j,a] + x2[b,i,a])
+ bout with x1 = x@W1.T + b1, x2 = x@W2.T + b2 (B=32, N=128, D=A=512).

Direct evaluation needs B*N*N*A = 268M tanh elements -> ACT-bound (~250us).
Instead approximate tanh(s) ~ sum_k b_k sin(w_k s) (K=10 freqs, two octave
lines {base*2^m}), so tanh(u+v) becomes sum_k b_k [sin_k(u)cos_k(v) +
cos_k(u)sin_k(v)] -- a rank-2K separable form. The NxN cross product then
collapses into TensorEngine matmuls over a (K,A) contraction and the
elementwise work drops to ~262K-element feature streams per core:

  ACT: base sin/cos per octave line (args |w0*y| <= pi/2, in Sin's valid
       range) and Square for sin^2 (both live in the trig_and_small table).
  DVE: frequency doubling s2=s*c (bf16 2x), cos via ts-dual 1-2*sigma^2,
       whose scalar slots also absorb Wout[a]*b_k weighting + cascade scale
       corrections; y staging PSUM->SBUF with bias add.
  PE : x1/x2 input matmuls + 320 accumulating [128p,127m,128n] matmuls
       contracting the feature dim into psum[b][i,j].

Sharding: data-parallel over batch across 8 cores (4 batches/core), weights
replicated, no collectives. Coefficients b_k fit offline (Gaussian-weighted
LS, ridge 1e-6); e2e rel err ~3.8e-3 (tolerance 2e-2).
"""
import sys
import numpy as np

if "/opt/trn_rl_repo" not in sys.path:
    sys.path.insert(0, "/opt/trn_rl_repo")

B, N, D, A = 32, 128, 512, 512
NCORES = 8
BPC = B // NCORES        # batches per core
TOK = BPC * N            # tokens per core (b,n flattened) = 512
KC = D // 128            # d contraction chunks
MC = A // 128            # a chunks
PI = float(np.pi)

# ---- offline Fourier fit of tanh: two octave lines ----
STRUCT = [(0.2038, 5), (0.28, 4)]   # (base freq, levels); freqs base*2^m
RIDGE = 1e-5


def _fit_coeffs():
    sg = np.linspace(-11, 11, 4001)
    w = np.exp(-sg ** 2 / 4.0) + 1e-5
    t = np.tanh(sg)
    freqs = np.concatenate([[bb * 2 ** m for m in range(L)] for bb, L in STRUCT])
    X = np.sin(np.outer(sg, freqs))
    G = X.T @ (X * w[:, None])
    r = X.T @ (t * w)
    bk = np.linalg.solve(G + RIDGE * np.eye(len(freqs)), r)
    return bk.astype(np.float64)


_BK = _fit_coeffs()

_CACHE = {}


def _build_nc():
    import concourse.bass as bass
    import concourse.bacc as bacc
    import concourse.mybir as mybir
    from concourse import tile

    f32 = mybir.dt.float32
    bf16 = mybir.dt.bfloat16
    AF = mybir.ActivationFunctionType
    ALU = mybir.AluOpType

    nc = bacc.Bacc(None, target_bir_lowering=False)

    # DRAM params. xT: [d', k*TOK + t]; wpk: [d', ((side*MC+c)*KC+k)*128 + j]
    big0 = nc.declare_dram_parameter("big0", [128, KC * TOK + MC * KC * 128], bf16, isOutput=False)
    big1 = nc.declare_dram_parameter("big1", [128, MC * KC * 128], bf16, isOutput=False)
    NWC = sum((4 if m == 0 else 8) for bb, L in STRUCT for m in range(L))
    wcoef = nc.declare_dram_parameter("wcoef", [128, NWC], f32, isOutput=False)
    # duplicated-pair bf16 w*coef columns for inner cw TT broadcast:
    # per (line, m<=L-2, chunk): value w_a*coef duplicated in adjacent cols
    NWD = sum(1 for bb, L in STRUCT for m in range(L - 1)) * MC * 2
    wdup = nc.declare_dram_parameter("wdup", [128, NWD], bf16, isOutput=False)
    boutp = nc.declare_dram_parameter("bout", [128, 1], f32, isOutput=False)
    NSB = len(STRUCT) * 2 * MC
    sinb = nc.declare_dram_parameter("sinb", [128, NSB], f32, isOutput=False)
    bias12 = nc.declare_dram_parameter("bias12", [128, 2 * MC], f32, isOutput=False)
    out = nc.declare_dram_parameter("out", [N - 1, BPC * N], f32, isOutput=True)

    with tile.TileContext(nc) as tc:
        with (
            tc.tile_pool(name="const", bufs=1) as cpool,
            tc.tile_pool(name="xw", bufs=1) as xwpool,
            tc.tile_pool(name="y", bufs=1) as ypool,
            tc.tile_pool(name="feat", bufs=1) as fpool,
            tc.tile_pool(name="stage", bufs=4) as stpool,
        ):
            # ---- input DMAs: one DMA covers x + side-0 weights ----
            H = MC * KC * 128
            b0sb = xwpool.tile([128, KC * TOK + H], bf16, tag="b0sb")
            nc.sync.dma_start(b0sb[:, :], big0[:, :])
            xsb = b0sb
            b1sb = xwpool.tile([128, H], bf16, tag="b1sb")
            nc.gpsimd.dma_start(b1sb[:, :], big1[:, :])
            NSB = len(STRUCT) * 2 * MC
            snb = cpool.tile([128, NSB], f32, tag="snb")
            nc.scalar.dma_start(snb[:, :], sinb[:, :])
            b12 = cpool.tile([128, 2 * MC], f32, tag="b12")
            nc.scalar.dma_start(b12[:, :], bias12[:, :])
            wco = cpool.tile([128, NWC], f32, tag="wco")
            nc.scalar.dma_start(wco[:, :], wcoef[:, :])
            NWD = sum(1 for bb, L in STRUCT for m in range(L - 1)) * MC * 2
            wdu = cpool.tile([128, NWD], bf16, tag="wdu")
            nc.scalar.dma_start(wdu[:, :], wdup[:, :])
            boutf = cpool.tile([128, 1], f32, tag="boutf")
            nc.scalar.dma_start(boutf[:, :], boutp[:, :])
            halfpi = cpool.tile([128, 1], f32, tag="halfpi")
            nc.gpsimd.memset(halfpi[:, :], PI / 2)

            def wslice(side, c, k):
                o = (c * KC + k) * 128
                if side == 0:
                    return b0sb[:, KC * TOK + o:KC * TOK + o + 128]
                return b1sb[:, o:o + 128]

            warm = cpool.tile([128, 128], bf16, tag="warm")
            nc.gpsimd.memset(warm[:, :], 0.25)
            with (
                tc.tile_pool(name="psF", bufs=1, space=bass.MemorySpace.PSUM) as psF,
                tc.tile_pool(name="psS", bufs=4, space=bass.MemorySpace.PSUM) as psS,
                tc.tile_pool(name="psX", bufs=2, space=bass.MemorySpace.PSUM) as psX,
                tc.tile_pool(name="psO", bufs=1, space=bass.MemorySpace.PSUM) as psO,
            ):
                fps = psF.tile([128, 128], f32, tag="psF")

                def filler(n, dep=None):
                    rhs = warm[:, :] if dep is None else dep[:, 0:128]
                    for _ in range(n):
                        nc.tensor.matmul(fps[:, :], warm[:, :], rhs,
                                         start=True, stop=True)
                # ---- x matmuls. side0 psums stay resident for direct Sin;
                # side1 staged to SBUF via ACT Identity (+bias) ----
                y1 = ypool.tile([128, MC * TOK], f32, tag="y1", name="y1")
                filler(6)
                ps0 = []
                for c in range(MC):
                    ps = psS.tile([128, TOK], f32, tag="psS", name=f"ps0_{c}")
                    for k in range(KC):
                        nc.tensor.matmul(ps[:, :], wslice(0, c, k),
                                         xsb[:, k * TOK:(k + 1) * TOK],
                                         start=(k == 0), stop=(k == KC - 1))
                    ps0.append(ps)
                for c in range(MC):
                    ps = psX.tile([128, TOK], f32, tag="psX", name=f"psx1_{c}")
                    for k in range(KC):
                        nc.tensor.matmul(ps[:, :], wslice(1, c, k),
                                         xsb[:, k * TOK:(k + 1) * TOK],
                                         start=(k == 0), stop=(k == KC - 1))
                    nc.vector.tensor_scalar(
                        y1[:, c * TOK:(c + 1) * TOK], ps[:, :],
                        b12[:, MC + c:MC + c + 1], None, ALU.add)

                pout_t = psO.tile([N - 1, BPC * N], f32, tag="psO", name="pout")
                pouts = [pout_t[:, b * N:(b + 1) * N] for b in range(BPC)]
                zro = cpool.tile([128, BPC * N], bf16, tag="zro")
                nc.gpsimd.memset(zro[:, :], 0.0)
                nc.tensor.matmul(pout_t[:, :], warm[:, 0:N - 1], zro[:, :],
                                 start=True, stop=False, skip_group_check=True)
                FW = MC * TOK

                st = {}       # (li, side, m) -> (s_tile, c_tile)
                sq = {}       # (li, side, m) -> sigma^2 tile
                mm_first = True
                dmaq = [nc.sync, nc.scalar, nc.sync, nc.scalar]

                # wcoef column offsets per (line, m)
                wc_off = []
                o = 0
                for li, (bb, L) in enumerate(STRUCT):
                    offs = []
                    for m in range(L):
                        offs.append(o)
                        o += 4 if m == 0 else 8
                    wc_off.append(offs)
                # wdup column offsets per (line, m<=L-2): MC*2 bf16 cols each
                wd_off = []
                o = 0
                for li, (bb, L) in enumerate(STRUCT):
                    offs = []
                    for m in range(L - 1):
                        offs.append(o)
                        o += MC * 2
                    wd_off.append(offs)

                def make_cw_tt(li, m, side, c_tile):
                    # cw = c_m * (w_a*coef) via TT-mult against dup-pair
                    # broadcast AP (keeps DVE 2x mode)
                    cw = fpool.tile([128, FW], bf16, tag=f"cw_{li}_{side}_p{m % 2}",
                                    name=f"cw{li}{side}{m}")
                    wap = wdu[:, :]
                    in1 = bass.AP(wap.tensor, wap.offset + wd_off[li][m],
                                  [[wap.ap[0][0], 128], [2, MC], [0, TOK // 2],
                                   [1, 2]])
                    cap = c_tile[:, :]
                    in0 = bass.AP(cap.tensor, cap.offset,
                                  [[cap.ap[0][0], 128], [TOK, MC], [2, TOK // 2],
                                   [1, 2]])
                    oap = cw[:, :]
                    o0 = bass.AP(oap.tensor, oap.offset,
                                 [[oap.ap[0][0], 128], [TOK, MC], [2, TOK // 2],
                                  [1, 2]])
                    nc.vector.tensor_tensor(o0, in0, in1, ALU.mult)
                    return cw

                def make_cw_leaf(li, m, side, sq_tile):
                    # leaf: cw = sigma^2*(-2 w coef/lam^2) + w coef (dual ts)
                    # as per-chunk tiles so final mms start on chunk 0
                    cws = []
                    for c in range(MC):
                        cwc = fpool.tile([128, TOK], bf16,
                                         tag=f"cwl_{li}_{side}_{c}",
                                         name=f"cwl{li}{side}{m}{c}")
                        col = wc_off[li][m] + 2 * c
                        nc.vector.tensor_scalar(
                            cwc[:, :],
                            sq_tile[:, c * TOK:(c + 1) * TOK],
                            wco[:, col:col + 1], wco[:, col + 1:col + 2],
                            ALU.mult, ALU.add)
                        cws.append(cwc)
                    return cws

                def emit_mms(li, m, cws, last):
                    # A: psum[i,j] += sum_a c2w[a,i]*s1[a,j]
                    # B: psum[i,j] += sum_a s2[a,i]*c1w[a,j]
                    nonlocal mm_first
                    s1_t = st[(li, 0, m)][0]
                    s2_t = st[(li, 1, m)][0]
                    c1w_t, c2w_t = cws

                    def cw_sl(t, c, lo, hi):
                        if isinstance(t, list):
                            return t[c][:, lo:hi]
                        return t[:, c * TOK + lo:c * TOK + hi]
                    if not last:
                        for c in range(MC):
                            for b in range(BPC):
                                t0 = c * TOK + b * N
                                nc.tensor.matmul(
                                    pouts[b],
                                    cw_sl(c2w_t, c, b * N + 1, b * N + N),
                                    s1_t[:, t0:t0 + N],
                                    start=False, stop=False,
                                    skip_group_check=True)
                                nc.tensor.matmul(
                                    pouts[b],
                                    s2_t[:, t0 + 1:t0 + N],
                                    cw_sl(c1w_t, c, b * N, b * N + N),
                                    start=False, stop=False,
                                    skip_group_check=True)
                        mm_first = False
                        return
                    # final pair: batch-major mms, stage into one wide tile,
                    # two big (>=4KB/partition-row) DMAs
                    stg = stpool.tile([N - 1, BPC * N], f32, tag="stg")
                    for b in range(BPC):
                        for c in range(MC):
                            t0 = c * TOK + b * N
                            nc.tensor.matmul(
                                pouts[b],
                                cw_sl(c2w_t, c, b * N + 1, b * N + N),
                                s1_t[:, t0:t0 + N],
                                start=False, stop=False,
                                skip_group_check=True)
                            nc.tensor.matmul(
                                pouts[b],
                                s2_t[:, t0 + 1:t0 + N],
                                cw_sl(c1w_t, c, b * N, b * N + N),
                                start=False,
                                stop=(c == MC - 1),
                                skip_group_check=True)
                        nc.scalar.activation(stg[:, b * N:(b + 1) * N],
                                             pouts[b], AF.Identity,
                                             bias=boutf[0:N - 1, 0:1])
                        if b == 1:
                            nc.sync.dma_start(out[:, 0:2 * N], stg[:, 0:2 * N])
                        elif b == BPC - 1:
                            nc.scalar.dma_start(out[:, 2 * N:], stg[:, 2 * N:])

                # ---- bases: per line, side0 direct from PSUM then side1
                # from y1, so ACT starts as soon as side-0 psums land ----
                for li, (bb, L) in enumerate(STRUCT):
                    for ph in range(2):
                        f0 = fpool.tile([128, FW], bf16,
                                        tag=("s" if ph == 0 else "c") + f"_{li}_0_p0",
                                        name=("s" if ph == 0 else "c") + f"{li}00")
                        for c in range(MC):
                            col = (li * 2 + ph) * MC + c
                            nc.scalar.activation(f0[:, c * TOK:(c + 1) * TOK],
                                                 ps0[c][:, :], AF.Sin,
                                                 bias=snb[:, col:col + 1],
                                                 scale=float(bb))
                        if ph == 0:
                            s0_0 = f0
                        else:
                            st[(li, 0, 0)] = (s0_0, f0)
                    s0 = fpool.tile([128, FW], bf16, tag=f"s_{li}_1_p0",
                                    name=f"s{li}10")
                    nc.scalar.activation(s0[:, :], y1[:, :], AF.Sin,
                                         bias=0.0, scale=float(bb))
                    c0 = fpool.tile([128, FW], bf16, tag=f"c_{li}_1_p0",
                                    name=f"c{li}10")
                    nc.scalar.activation(c0[:, :], y1[:, :], AF.Sin,
                                         bias=halfpi[:, 0:1], scale=float(bb))
                    st[(li, 1, 0)] = (s0, c0)
                for li, (bb, L) in enumerate(STRUCT):
                    for side in range(2):
                        t = fpool.tile([128, FW], bf16, tag=f"sq_{li}_{side}",
                                       name=f"sq{li}{side}0")
                        nc.scalar.activation(t[:, :], st[(li, side, 0)][0][:, :],
                                             AF.Square)
                        sq[(li, side, 0)] = t
                for li, (bb, L) in enumerate(STRUCT):
                    cws = [make_cw_tt(li, 0, side, st[(li, side, 0)][1])
                           for side in range(2)]
                    emit_mms(li, 0, cws, last=False)
                    filler(2, dep=st[(li, 0, 0)][0])

                # ---- staggered level schedule ----
                steps = []
                Lmax = max(L for _, L in STRUCT)
                for m in range(1, Lmax):
                    for li, (bb, L) in enumerate(STRUCT):
                        if m < L:
                            steps.append((li, m))
                for si, (li, m) in enumerate(steps):
                    bb, L = STRUCT[li]
                    lam_prev = 0.5 ** (m - 1)
                    is_last = si == len(steps) - 1
                    cms = [None, None]
                    if m <= L - 2:
                        for side in range(2):
                            cm = fpool.tile([128, FW], bf16,
                                            tag=f"c_{li}_{side}_p{m % 2}",
                                            name=f"c{li}{side}{m}")
                            nc.vector.tensor_scalar(
                                cm[:, :], sq[(li, side, m - 1)][:, :],
                                float(-2.0 / lam_prev ** 2), 1.0,
                                ALU.mult, ALU.add)
                            cms[side] = cm

                    def make_cw_for(side):
                        if m <= L - 2:
                            return make_cw_tt(li, m, side, cms[side])
                        return make_cw_leaf(li, m, side, sq[(li, side, m - 1)])

                    def make_s(side):
                        sp, cp = st[(li, side, m - 1)]
                        sm = fpool.tile([128, FW], bf16,
                                        tag=f"s_{li}_{side}_p{m % 2}",
                                        name=f"s{li}{side}{m}")
                        nc.vector.tensor_tensor(sm[:, :], sp[:, :], cp[:, :],
                                                ALU.mult)
                        return sm

                    if m <= L - 2:
                        cw1 = make_cw_for(1)
                        s1m = make_s(0)
                        cw0 = make_cw_for(0)
                        s2m = make_s(1)
                    else:
                        s1m = make_s(0)
                        s2m = make_s(1)
                        filler(6, dep=s1m)
                        cw1 = make_cw_for(1)
                        cw0 = make_cw_for(0)
                    st[(li, 0, m)] = (s1m, cms[0])
                    st[(li, 1, m)] = (s2m, cms[1])
                    if m <= L - 2:
                        for side, smt in ((0, s1m), (1, s2m)):
                            t = fpool.tile([128, FW], bf16, tag=f"sq_{li}_{side}",
                                           name=f"sq{li}{side}{m}")
                            nc.scalar.activation(t[:, :], smt[:, :], AF.Square)
                            sq[(li, side, m)] = t
                    emit_mms(li, m, [cw0, cw1], last=is_last)
                    if not is_last:
                        filler(2, dep=st[(li, 0, m - 1)][0])

    nc.finalize()
    return nc


def _get_nc():
    if "nc" not in _CACHE:
        _CACHE["nc"] = _build_nc()
    return _CACHE["nc"]


def _prep_in_maps(x, W1, b1, W2, b2, Wout, bout):
    import ml_dtypes
    f = np.float32
    bfd = ml_dtypes.bfloat16
    x = np.asarray(x, f)
    W1 = np.asarray(W1, f)
    W2 = np.asarray(W2, f)
    Wo = np.asarray(Wout, f)

    # weights: wpk[d', ((side*MC+c)*KC+k)*128 + j] = Wside[c*128+j, k*128+d']
    def pack_w(W):
        # [MC, 128j, KC, 128d'] -> transpose to [128d', MC, KC, 128j]
        t = W.reshape(MC, 128, KC, 128).transpose(3, 0, 2, 1)
        return np.ascontiguousarray(t.reshape(128, MC * KC * 128)).astype(f)

    w1p = pack_w(W1)
    w2p = pack_w(W2)
    bias12 = np.ascontiguousarray(
        np.concatenate([np.asarray(b1, f).reshape(MC, 128).T,
                        np.asarray(b2, f).reshape(MC, 128).T], axis=1))
    NSB = len(STRUCT) * 2 * MC
    snb = np.zeros((128, NSB), f)
    b1c = np.asarray(b1, f).reshape(MC, 128).T
    for li, (bb, L) in enumerate(STRUCT):
        for ph in range(2):
            for c in range(MC):
                snb[:, (li * 2 + ph) * MC + c] = bb * b1c[:, c] + (0.0 if ph == 0 else PI / 2)

    # wcoef table
    NWC = sum((4 if m == 0 else 8) for bb, L in STRUCT for m in range(L))
    wco = np.zeros((128, NWC), f)
    o = 0
    ki = 0
    for bb, L in STRUCT:
        for m in range(L):
            lam = 0.5 ** m
            coef = _BK[ki + m] / lam
            if m == 0:
                for c in range(MC):
                    wco[:, o + c] = Wo[c * 128:(c + 1) * 128] * coef
                o += 4
            else:
                lam_prev = 0.5 ** (m - 1)
                for c in range(MC):
                    wchunk = Wo[c * 128:(c + 1) * 128]
                    wco[:, o + 2 * c] = wchunk * (-2.0 * coef / lam_prev ** 2)
                    wco[:, o + 2 * c + 1] = wchunk * coef
                o += 8
        ki += L
    bov = np.full((128, 1), np.asarray(bout, f).reshape(()), f)

    # duplicated-pair bf16 table for inner cw TT: per (line, m<=L-2, chunk)
    NWD = sum(1 for bb, L in STRUCT for m in range(L - 1)) * MC * 2
    wdu = np.zeros((128, NWD), f)
    o = 0
    ki = 0
    for bb, L in STRUCT:
        for m in range(L - 1):
            coef = _BK[ki + m] / (0.5 ** m)
            for c in range(MC):
                v = Wo[c * 128:(c + 1) * 128] * coef
                wdu[:, o + 2 * c] = v
                wdu[:, o + 2 * c + 1] = v
            o += MC * 2
        ki += L
    wdu = wdu.astype(bfd)

    in_maps = []
    for ci in range(NCORES):
        xs = x[ci * BPC:(ci + 1) * BPC]          # [BPC, N, D]
        xt = xs.transpose(2, 0, 1).reshape(D, TOK)   # [D, TOK]
        # -> [128, KC*TOK]: xTi[d', k*TOK + t] = xt[k*128+d', t]
        xTi = np.ascontiguousarray(
            xt.reshape(KC, 128, TOK).transpose(1, 0, 2).reshape(128, KC * TOK))
        in_maps.append({
            "big0": np.ascontiguousarray(np.concatenate([xTi.astype(np.float32),
                                                         w1p], axis=1)).astype(bfd),
            "big1": w2p.astype(bfd), "bias12": bias12, "sinb": snb,
            "wcoef": wco, "bout": bov, "wdup": wdu,
        })
    return in_maps


def _run(x, W1, b1, W2, b2, Wout, bout, trace=False):
    from concourse.bass_utils import run_bass_kernel_spmd

    nc = _get_nc()
    in_maps = _prep_in_maps(x, W1, b1, W2, b2, Wout, bout)
    res = run_bass_kernel_spmd(nc, in_maps, core_ids=list(range(NCORES)),
                               trace=trace)
    outs = [np.asarray(res.results[ci]["out"]).reshape(N - 1, BPC, N)
            .transpose(1, 0, 2) for ci in range(NCORES)]
    full = np.concatenate(outs, axis=0).astype(np.float32)
    return full, res


def kernel(x, W1, b1, W2, b2, Wout, bout):
    full, _ = _run(x, W1, b1, W2, b2, Wout, bout, trace=False)
    return full


# revision 30
# speedup vs baseline: 1.0483x; 1.0117x over previous
"""Trainium2 Bass kernel: AdditiveAttention scoring head via separable
Fourier-feature expansion.

Reference computes out[b,i-1,j] = sum_a Wout[a]*tanh(x1[b,j,a] + x2[b,i,a])
+ bout with x1 = x@W1.T + b1, x2 = x@W2.T + b2 (B=32, N=128, D=A=512).

Direct evaluation needs B*N*N*A = 268M tanh elements -> ACT-bound (~250us).
Instead approximate tanh(s) ~ sum_k b_k sin(w_k s) (K=10 freqs, two octave
lines {base*2^m}), so tanh(u+v) becomes sum_k b_k [sin_k(u)cos_k(v) +
cos_k(u)sin_k(v)] -- a rank-2K separable form. The NxN cross product then
collapses into TensorEngine matmuls over a (K,A) contraction and the
elementwise work drops to ~262K-element feature streams per core:

  ACT: base sin/cos per octave line (args |w0*y| <= pi/2, in Sin's valid
       range) and Square for sin^2 (both live in the trig_and_small table).
  DVE: frequency doubling s2=s*c (bf16 2x), cos via ts-dual 1-2*sigma^2,
       whose scalar slots also absorb Wout[a]*b_k weighting + cascade scale
       corrections; y staging PSUM->SBUF with bias add.
  PE : x1/x2 input matmuls + 320 accumulating [128p,127m,128n] matmuls
       contracting the feature dim into psum[b][i,j].

Sharding: data-parallel over batch across 8 cores (4 batches/core), weights
replicated, no collectives. Coefficients b_k fit offline (Gaussian-weighted
LS, ridge 1e-6); e2e rel err ~3.8e-3 (tolerance 2e-2).
"""
import sys
import numpy as np

if "/opt/trn_rl_repo" not in sys.path:
    sys.path.insert(0, "/opt/trn_rl_repo")

B, N, D, A = 32, 128, 512, 512
NCORES = 8
BPC = B // NCORES        # batches per core
TOK = BPC * N            # tokens per core (b,n flattened) = 512
KC = D // 128            # d contraction chunks
MC = A // 128            # a chunks
PI = float(np.pi)

# ---- offline Fourier fit of tanh: two octave lines ----
STRUCT = [(0.2038, 5), (0.28, 4)]   # (base freq, levels); freqs base*2^m
RIDGE = 1e-5


def _fit_coeffs():
    sg = np.linspace(-11, 11, 4001)
    w = np.exp(-sg ** 2 / 4.0) + 1e-5
    t = np.tanh(sg)
    freqs = np.concatenate([[bb * 2 ** m for m in range(L)] for bb, L in STRUCT])
    X = np.sin(np.outer(sg, freqs))
    G = X.T @ (X * w[:, None])
    r = X.T @ (t * w)
    bk = np.linalg.solve(G + RIDGE * np.eye(len(freqs)), r)
    return bk.astype(np.float64)


_BK = _fit_coeffs()

_CACHE = {}


def _build_nc():
    import concourse.bass as bass
    import concourse.bacc as bacc
    import concourse.mybir as mybir
    from concourse import tile

    f32 = mybir.dt.float32
    bf16 = mybir.dt.bfloat16
    AF = mybir.ActivationFunctionType
    ALU = mybir.AluOpType

    nc = bacc.Bacc(None, target_bir_lowering=False)

    # DRAM params. xT: [d', k*TOK + t]; wpk: [d', ((side*MC+c)*KC+k)*128 + j]
    big0 = nc.declare_dram_parameter("big0", [128, KC * TOK + MC * KC * 128], bf16, isOutput=False)
    big1 = nc.declare_dram_parameter("big1", [128, MC * KC * 128], bf16, isOutput=False)
    NWC = sum((4 if m == 0 else 8) for bb, L in STRUCT for m in range(L))
    wcoef = nc.declare_dram_parameter("wcoef", [128, NWC], f32, isOutput=False)
    # duplicated-pair bf16 w*coef columns for inner cw TT broadcast:
    # per (line, m<=L-2, chunk): value w_a*coef duplicated in adjacent cols
    NWD = sum(1 for bb, L in STRUCT for m in range(L - 1)) * MC * 2
    wdup = nc.declare_dram_parameter("wdup", [128, NWD], bf16, isOutput=False)
    boutp = nc.declare_dram_parameter("bout", [128, 1], f32, isOutput=False)
    NSB = len(STRUCT) * 2 * MC
    sinb = nc.declare_dram_parameter("sinb", [128, NSB], f32, isOutput=False)
    bias12 = nc.declare_dram_parameter("bias12", [128, 2 * MC], f32, isOutput=False)
    out = nc.declare_dram_parameter("out", [N - 1, BPC * N], f32, isOutput=True)

    with tile.TileContext(nc) as tc:
        with (
            tc.tile_pool(name="const", bufs=1) as cpool,
            tc.tile_pool(name="xw", bufs=1) as xwpool,
            tc.tile_pool(name="y", bufs=1) as ypool,
            tc.tile_pool(name="feat", bufs=1) as fpool,
            tc.tile_pool(name="stage", bufs=4) as stpool,
        ):
            # ---- input DMAs: one DMA covers x + side-0 weights ----
            H = MC * KC * 128
            b0sb = xwpool.tile([128, KC * TOK + H], bf16, tag="b0sb")
            nc.sync.dma_start(b0sb[:, :], big0[:, :])
            xsb = b0sb
            b1sb = xwpool.tile([128, H], bf16, tag="b1sb")
            nc.gpsimd.dma_start(b1sb[:, :], big1[:, :])
            NSB = len(STRUCT) * 2 * MC
            snb = cpool.tile([128, NSB], f32, tag="snb")
            nc.scalar.dma_start(snb[:, :], sinb[:, :])
            b12 = cpool.tile([128, 2 * MC], f32, tag="b12")
            nc.scalar.dma_start(b12[:, :], bias12[:, :])
            wco = cpool.tile([128, NWC], f32, tag="wco")
            nc.scalar.dma_start(wco[:, :], wcoef[:, :])
            NWD = sum(1 for bb, L in STRUCT for m in range(L - 1)) * MC * 2
            wdu = cpool.tile([128, NWD], bf16, tag="wdu")
            nc.scalar.dma_start(wdu[:, :], wdup[:, :])
            boutf = cpool.tile([128, 1], f32, tag="boutf")
            nc.scalar.dma_start(boutf[:, :], boutp[:, :])
            halfpi = cpool.tile([128, 1], f32, tag="halfpi")
            nc.gpsimd.memset(halfpi[:, :], PI / 2)

            def wslice(side, c, k):
                o = (c * KC + k) * 128
                if side == 0:
                    return b0sb[:, KC * TOK + o:KC * TOK + o + 128]
                return b1sb[:, o:o + 128]

            warm = cpool.tile([128, 128], bf16, tag="warm")
            nc.gpsimd.memset(warm[:, :], 0.25)
            with (
                tc.tile_pool(name="psF", bufs=1, space=bass.MemorySpace.PSUM) as psF,
                tc.tile_pool(name="psS", bufs=4, space=bass.MemorySpace.PSUM) as psS,
                tc.tile_pool(name="psX", bufs=2, space=bass.MemorySpace.PSUM) as psX,
                tc.tile_pool(name="psO", bufs=1, space=bass.MemorySpace.PSUM) as psO,
            ):
                fps = psF.tile([128, 128], f32, tag="psF")

                def filler(n, dep=None):
                    rhs = warm[:, :] if dep is None else dep[:, 0:128]
                    for _ in range(n):
                        nc.tensor.matmul(fps[:, :], warm[:, :], rhs,
                                         start=True, stop=True)
                # ---- x matmuls. side0 psums stay resident for direct Sin;
                # side1 staged to SBUF via ACT Identity (+bias) ----
                y1 = ypool.tile([128, MC * TOK], f32, tag="y1", name="y1")
                ps0 = []
                for c in range(MC):
                    ps = psS.tile([128, TOK], f32, tag="psS", name=f"ps0_{c}")
                    for k in range(KC):
                        nc.tensor.matmul(ps[:, :], wslice(0, c, k),
                                         xsb[:, k * TOK:(k + 1) * TOK],
                                         start=(k == 0), stop=(k == KC - 1))
                    ps0.append(ps)
                for c in range(MC):
                    ps = psX.tile([128, TOK], f32, tag="psX", name=f"psx1_{c}")
                    for k in range(KC):
                        nc.tensor.matmul(ps[:, :], wslice(1, c, k),
                                         xsb[:, k * TOK:(k + 1) * TOK],
                                         start=(k == 0), stop=(k == KC - 1))
                    nc.vector.tensor_scalar(
                        y1[:, c * TOK:(c + 1) * TOK], ps[:, :],
                        b12[:, MC + c:MC + c + 1], None, ALU.add)

                pout_t = psO.tile([N - 1, BPC * N], f32, tag="psO", name="pout")
                pouts = [pout_t[:, b * N:(b + 1) * N] for b in range(BPC)]
                zro = cpool.tile([128, BPC * N], bf16, tag="zro")
                nc.gpsimd.memset(zro[:, :], 0.0)
                nc.tensor.matmul(pout_t[:, :], warm[:, 0:N - 1], zro[:, :],
                                 start=True, stop=False, skip_group_check=True)
                FW = MC * TOK

                st = {}       # (li, side, m) -> (s_tile, c_tile)
                sq = {}       # (li, side, m) -> sigma^2 tile
                mm_first = True
                dmaq = [nc.sync, nc.scalar, nc.sync, nc.scalar]

                # wcoef column offsets per (line, m)
                wc_off = []
                o = 0
                for li, (bb, L) in enumerate(STRUCT):
                    offs = []
                    for m in range(L):
                        offs.append(o)
                        o += 4 if m == 0 else 8
                    wc_off.append(offs)
                # wdup column offsets per (line, m<=L-2): MC*2 bf16 cols each
                wd_off = []
                o = 0
                for li, (bb, L) in enumerate(STRUCT):
                    offs = []
                    for m in range(L - 1):
                        offs.append(o)
                        o += MC * 2
                    wd_off.append(offs)

                def make_cw_tt(li, m, side, c_tile):
                    # cw = c_m * (w_a*coef) via TT-mult against dup-pair
                    # broadcast AP (keeps DVE 2x mode)
                    cw = fpool.tile([128, FW], bf16, tag=f"cw_{li}_{side}_p{m % 2}",
                                    name=f"cw{li}{side}{m}")
                    wap = wdu[:, :]
                    in1 = bass.AP(wap.tensor, wap.offset + wd_off[li][m],
                                  [[wap.ap[0][0], 128], [2, MC], [0, TOK // 2],
                                   [1, 2]])
                    cap = c_tile[:, :]
                    in0 = bass.AP(cap.tensor, cap.offset,
                                  [[cap.ap[0][0], 128], [TOK, MC], [2, TOK // 2],
                                   [1, 2]])
                    oap = cw[:, :]
                    o0 = bass.AP(oap.tensor, oap.offset,
                                 [[oap.ap[0][0], 128], [TOK, MC], [2, TOK // 2],
                                  [1, 2]])
                    nc.vector.tensor_tensor(o0, in0, in1, ALU.mult)
                    return cw

                def make_cw_leaf(li, m, side, sq_tile):
                    # leaf: cw = sigma^2*(-2 w coef/lam^2) + w coef (dual ts)
                    # as per-chunk tiles so final mms start on chunk 0
                    cws = []
                    for c in range(MC):
                        cwc = fpool.tile([128, TOK], bf16,
                                         tag=f"cwl_{li}_{side}_{c}",
                                         name=f"cwl{li}{side}{m}{c}")
                        col = wc_off[li][m] + 2 * c
                        nc.vector.tensor_scalar(
                            cwc[:, :],
                            sq_tile[:, c * TOK:(c + 1) * TOK],
                            wco[:, col:col + 1], wco[:, col + 1:col + 2],
                            ALU.mult, ALU.add)
                        cws.append(cwc)
                    return cws

                def emit_mms(li, m, cws, last):
                    # A: psum[i,j] += sum_a c2w[a,i]*s1[a,j]
                    # B: psum[i,j] += sum_a s2[a,i]*c1w[a,j]
                    nonlocal mm_first
                    s1_t = st[(li, 0, m)][0]
                    s2_t = st[(li, 1, m)][0]
                    c1w_t, c2w_t = cws

                    def cw_sl(t, c, lo, hi):
                        if isinstance(t, list):
                            return t[c][:, lo:hi]
                        return t[:, c * TOK + lo:c * TOK + hi]
                    if not last:
                        for c in range(MC):
                            for b in range(BPC):
                                t0 = c * TOK + b * N
                                nc.tensor.matmul(
                                    pouts[b],
                                    cw_sl(c2w_t, c, b * N + 1, b * N + N),
                                    s1_t[:, t0:t0 + N],
                                    start=False, stop=False,
                                    skip_group_check=True)
                                nc.tensor.matmul(
                                    pouts[b],
                                    s2_t[:, t0 + 1:t0 + N],
                                    cw_sl(c1w_t, c, b * N, b * N + N),
                                    start=False, stop=False,
                                    skip_group_check=True)
                        mm_first = False
                        return
                    # final pair: batch-major mms, stage into one wide tile,
                    # two big (>=4KB/partition-row) DMAs
                    stg = stpool.tile([N - 1, BPC * N], f32, tag="stg")
                    for b in range(BPC):
                        for c in range(MC):
                            t0 = c * TOK + b * N
                            nc.tensor.matmul(
                                pouts[b],
                                cw_sl(c2w_t, c, b * N + 1, b * N + N),
                                s1_t[:, t0:t0 + N],
                                start=False, stop=False,
                                skip_group_check=True)
                            nc.tensor.matmul(
                                pouts[b],
                                s2_t[:, t0 + 1:t0 + N],
                                cw_sl(c1w_t, c, b * N, b * N + N),
                                start=False,
                                stop=(c == MC - 1),
                                skip_group_check=True)
                        nc.scalar.activation(stg[:, b * N:(b + 1) * N],
                                             pouts[b], AF.Identity,
                                             bias=boutf[0:N - 1, 0:1])
                        if b == 1:
                            nc.sync.dma_start(out[:, 0:2 * N], stg[:, 0:2 * N])
                        elif b == BPC - 1:
                            nc.scalar.dma_start(out[:, 2 * N:], stg[:, 2 * N:])

                # ---- bases: per line, side0 direct from PSUM then side1
                # from y1, so ACT starts as soon as side-0 psums land ----
                for li, (bb, L) in enumerate(STRUCT):
                    for ph in range(2):
                        f0 = fpool.tile([128, FW], bf16,
                                        tag=("s" if ph == 0 else "c") + f"_{li}_0_p0",
                                        name=("s" if ph == 0 else "c") + f"{li}00")
                        for c in range(MC):
                            col = (li * 2 + ph) * MC + c
                            nc.scalar.activation(f0[:, c * TOK:(c + 1) * TOK],
                                                 ps0[c][:, :], AF.Sin,
                                                 bias=snb[:, col:col + 1],
                                                 scale=float(bb))
                        if ph == 0:
                            s0_0 = f0
                        else:
                            st[(li, 0, 0)] = (s0_0, f0)
                    s0 = fpool.tile([128, FW], bf16, tag=f"s_{li}_1_p0",
                                    name=f"s{li}10")
                    nc.scalar.activation(s0[:, :], y1[:, :], AF.Sin,
                                         bias=0.0, scale=float(bb))
                    c0 = fpool.tile([128, FW], bf16, tag=f"c_{li}_1_p0",
                                    name=f"c{li}10")
                    nc.scalar.activation(c0[:, :], y1[:, :], AF.Sin,
                                         bias=halfpi[:, 0:1], scale=float(bb))
                    st[(li, 1, 0)] = (s0, c0)
                for li, (bb, L) in enumerate(STRUCT):
                    for side in range(2):
                        t = fpool.tile([128, FW], bf16, tag=f"sq_{li}_{side}",
                                       name=f"sq{li}{side}0")
                        nc.scalar.activation(t[:, :], st[(li, side, 0)][0][:, :],
                                             AF.Square)
                        sq[(li, side, 0)] = t
                for li, (bb, L) in enumerate(STRUCT):
                    cws = [make_cw_tt(li, 0, side, st[(li, side, 0)][1])
                           for side in range(2)]
                    emit_mms(li, 0, cws, last=False)
                    filler(2, dep=st[(li, 0, 0)][0])

                # ---- staggered level schedule ----
                steps = []
                Lmax = max(L for _, L in STRUCT)
                for m in range(1, Lmax):
                    for li, (bb, L) in enumerate(STRUCT):
                        if m < L:
                            steps.append((li, m))
                for si, (li, m) in enumerate(steps):
                    bb, L = STRUCT[li]
                    lam_prev = 0.5 ** (m - 1)
                    is_last = si == len(steps) - 1
                    cms = [None, None]
                    if m <= L - 2:
                        for side in range(2):
                            cm = fpool.tile([128, FW], bf16,
                                            tag=f"c_{li}_{side}_p{m % 2}",
                                            name=f"c{li}{side}{m}")
                            nc.vector.tensor_scalar(
                                cm[:, :], sq[(li, side, m - 1)][:, :],
                                float(-2.0 / lam_prev ** 2), 1.0,
                                ALU.mult, ALU.add)
                            cms[side] = cm

                    def make_cw_for(side):
                        if m <= L - 2:
                            return make_cw_tt(li, m, side, cms[side])
                        return make_cw_leaf(li, m, side, sq[(li, side, m - 1)])

                    def make_s(side):
                        sp, cp = st[(li, side, m - 1)]
                        sm = fpool.tile([128, FW], bf16,
                                        tag=f"s_{li}_{side}_p{m % 2}",
                                        name=f"s{li}{side}{m}")
                        nc.vector.tensor_tensor(sm[:, :], sp[:, :], cp[:, :],
                                                ALU.mult)
                        return sm

                    if m <= L - 2:
                        cw1 = make_cw_for(1)
                        s1m = make_s(0)
                        cw0 = make_cw_for(0)
                        s2m = make_s(1)
                    else:
                        s1m = make_s(0)
                        s2m = make_s(1)
                        filler(6, dep=s1m)
                        cw1 = make_cw_for(1)
                        cw0 = make_cw_for(0)
                    st[(li, 0, m)] = (s1m, cms[0])
                    st[(li, 1, m)] = (s2m, cms[1])
                    if m <= L - 2:
                        for side, smt in ((0, s1m), (1, s2m)):
                            t = fpool.tile([128, FW], bf16, tag=f"sq_{li}_{side}",
                                           name=f"sq{li}{side}{m}")
                            nc.scalar.activation(t[:, :], smt[:, :], AF.Square)
                            sq[(li, side, m)] = t
                    emit_mms(li, m, [cw0, cw1], last=is_last)
                    if not is_last:
                        filler(2, dep=st[(li, 0, m - 1)][0])

    nc.finalize()
    return nc


def _get_nc():
    if "nc" not in _CACHE:
        _CACHE["nc"] = _build_nc()
    return _CACHE["nc"]


def _prep_in_maps(x, W1, b1, W2, b2, Wout, bout):
    import ml_dtypes
    f = np.float32
    bfd = ml_dtypes.bfloat16
    x = np.asarray(x, f)
    W1 = np.asarray(W1, f)
    W2 = np.asarray(W2, f)
    Wo = np.asarray(Wout, f)

    # weights: wpk[d', ((side*MC+c)*KC+k)*128 + j] = Wside[c*128+j, k*128+d']
    def pack_w(W):
        # [MC, 128j, KC, 128d'] -> transpose to [128d', MC, KC, 128j]
        t = W.reshape(MC, 128, KC, 128).transpose(3, 0, 2, 1)
        return np.ascontiguousarray(t.reshape(128, MC * KC * 128)).astype(f)

    w1p = pack_w(W1)
    w2p = pack_w(W2)
    bias12 = np.ascontiguousarray(
        np.concatenate([np.asarray(b1, f).reshape(MC, 128).T,
                        np.asarray(b2, f).reshape(MC, 128).T], axis=1))
    NSB = len(STRUCT) * 2 * MC
    snb = np.zeros((128, NSB), f)
    b1c = np.asarray(b1, f).reshape(MC, 128).T
    for li, (bb, L) in enumerate(STRUCT):
        for ph in range(2):
            for c in range(MC):
                snb[:, (li * 2 + ph) * MC + c] = bb * b1c[:, c] + (0.0 if ph == 0 else PI / 2)

    # wcoef table
    NWC = sum((4 if m == 0 else 8) for bb, L in STRUCT for m in range(L))
    wco = np.zeros((128, NWC), f)
    o = 0
    ki = 0
    for bb, L in STRUCT:
        for m in range(L):
            lam = 0.5 ** m
            coef = _BK[ki + m] / lam
            if m == 0:
                for c in range(MC):
                    wco[:, o + c] = Wo[c * 128:(c + 1) * 128] * coef
                o += 4
            else:
                lam_prev = 0.5 ** (m - 1)
                for c in range(MC):
                    wchunk = Wo[c * 128:(c + 1) * 128]
                    wco[:, o + 2 * c] = wchunk * (-2.0 * coef / lam_prev ** 2)
                    wco[:, o + 2 * c + 1] = wchunk * coef
                o += 8
        ki += L
    bov = np.full((128, 1), np.asarray(bout, f).reshape(()), f)

    # duplicated-pair bf16 table for inner cw TT: per (line, m<=L-2, chunk)
    NWD = sum(1 for bb, L in STRUCT for m in range(L - 1)) * MC * 2
    wdu = np.zeros((128, NWD), f)
    o = 0
    ki = 0
    for bb, L in STRUCT:
        for m in range(L - 1):
            coef = _BK[ki + m] / (0.5 ** m)
            for c in range(MC):
                v = Wo[c * 128:(c + 1) * 128] * coef
                wdu[:, o + 2 * c] = v
                wdu[:, o + 2 * c + 1] = v
            o += MC * 2
        ki += L
    wdu = wdu.astype(bfd)

    in_maps = []
    for ci in range(NCORES):
        xs = x[ci * BPC:(ci + 1) * BPC]          # [BPC, N, D]
        xt = xs.transpose(2, 0, 1).reshape(D, TOK)   # [D, TOK]
        # -> [128, KC*TOK]: xTi[d', k*TOK + t] = xt[k*128+d', t]
        xTi = np.ascontiguousarray(
            xt.reshape(KC, 128, TOK).transpose(1, 0, 2).reshape(128, KC * TOK))
        in_maps.append({
            "big0": np.ascontiguousarray(np.concatenate([xTi.astype(np.float32),
                                                         w1p], axis=1)).astype(bfd),
            "big1": w2p.astype(bfd), "bias12": bias12, "sinb": snb,
            "wcoef": wco, "bout": bov, "wdup": wdu,
        })
    return in_maps


def _run(x, W1, b1, W2, b2, Wout, bout, trace=False):
    from concourse.bass_utils import run_bass_kernel_spmd

    nc = _get_nc()
    in_maps = _prep_in_maps(x, W1, b1, W2, b2, Wout, bout)
    res = run_bass_kernel_spmd(nc, in_maps, core_ids=list(range(NCORES)),
                               trace=trace)
    outs = [np.asarray(res.results[ci]["out"]).reshape(N - 1, BPC, N)
            .transpose(1, 0, 2) for ci in range(NCORES)]
    full = np.concatenate(outs, axis=0).astype(np.float32)
    return full, res


def kernel(x, W1, b1, W2, b2, Wout, bout):
    full, _ = _run(x, W1, b1, W2, b2, Wout, bout, trace=False)
    return full


# revision 31
# speedup vs baseline: 1.0862x; 1.0361x over previous
"""Trainium2 Bass kernel: AdditiveAttention scoring head via separable
Fourier-feature expansion.

Reference computes out[b,i-1,j] = sum_a Wout[a]*tanh(x1[b,j,a] + x2[b,i,a])
+ bout with x1 = x@W1.T + b1, x2 = x@W2.T + b2 (B=32, N=128, D=A=512).

Direct evaluation needs B*N*N*A = 268M tanh elements -> ACT-bound (~250us).
Instead approximate tanh(s) ~ sum_k b_k sin(w_k s) (K=10 freqs, two octave
lines {base*2^m}), so tanh(u+v) becomes sum_k b_k [sin_k(u)cos_k(v) +
cos_k(u)sin_k(v)] -- a rank-2K separable form. The NxN cross product then
collapses into TensorEngine matmuls over a (K,A) contraction and the
elementwise work drops to ~262K-element feature streams per core:

  ACT: base sin/cos per octave line (args |w0*y| <= pi/2, in Sin's valid
       range) and Square for sin^2 (both live in the trig_and_small table).
  DVE: frequency doubling s2=s*c (bf16 2x), cos via ts-dual 1-2*sigma^2,
       whose scalar slots also absorb Wout[a]*b_k weighting + cascade scale
       corrections; y staging PSUM->SBUF with bias add.
  PE : x1/x2 input matmuls + 320 accumulating [128p,127m,128n] matmuls
       contracting the feature dim into psum[b][i,j].

Sharding: data-parallel over batch across 8 cores (4 batches/core), weights
replicated, no collectives. Coefficients b_k fit offline (Gaussian-weighted
LS, ridge 1e-6); e2e rel err ~3.8e-3 (tolerance 2e-2).
"""
import sys
import numpy as np

if "/opt/trn_rl_repo" not in sys.path:
    sys.path.insert(0, "/opt/trn_rl_repo")

B, N, D, A = 32, 128, 512, 512
NCORES = 8
BPC = B // NCORES        # batches per core
TOK = BPC * N            # tokens per core (b,n flattened) = 512
KC = D // 128            # d contraction chunks
MC = A // 128            # a chunks
PI = float(np.pi)

# ---- offline Fourier fit of tanh: two octave lines ----
STRUCT = [(0.2038, 5), (0.28, 4)]   # (base freq, levels); freqs base*2^m
RIDGE = 1e-5


def _fit_coeffs():
    sg = np.linspace(-11, 11, 4001)
    w = np.exp(-sg ** 2 / 4.0) + 1e-5
    t = np.tanh(sg)
    freqs = np.concatenate([[bb * 2 ** m for m in range(L)] for bb, L in STRUCT])
    X = np.sin(np.outer(sg, freqs))
    G = X.T @ (X * w[:, None])
    r = X.T @ (t * w)
    bk = np.linalg.solve(G + RIDGE * np.eye(len(freqs)), r)
    return bk.astype(np.float64)


_BK = _fit_coeffs()

_CACHE = {}


def _build_nc():
    import concourse.bass as bass
    import concourse.bacc as bacc
    import concourse.mybir as mybir
    from concourse import tile

    f32 = mybir.dt.float32
    bf16 = mybir.dt.bfloat16
    AF = mybir.ActivationFunctionType
    ALU = mybir.AluOpType

    nc = bacc.Bacc(None, target_bir_lowering=False)

    # DRAM params. xT: [d', k*TOK + t]; wpk: [d', ((side*MC+c)*KC+k)*128 + j]
    big0 = nc.declare_dram_parameter("big0", [128, KC * TOK + MC * KC * 128], bf16, isOutput=False)
    big1 = nc.declare_dram_parameter("big1", [128, MC * KC * 128], bf16, isOutput=False)
    NWC = sum((4 if m == 0 else 8) for bb, L in STRUCT for m in range(L))
    wcoef = nc.declare_dram_parameter("wcoef", [128, NWC], f32, isOutput=False)
    # duplicated-pair bf16 w*coef columns for inner cw TT broadcast:
    # per (line, m<=L-2, chunk): value w_a*coef duplicated in adjacent cols
    NWD = sum(1 for bb, L in STRUCT for m in range(L - 1)) * MC * 2
    wdup = nc.declare_dram_parameter("wdup", [128, NWD], bf16, isOutput=False)
    boutp = nc.declare_dram_parameter("bout", [128, 1], f32, isOutput=False)
    NSB = len(STRUCT) * 2 * MC
    sinb = nc.declare_dram_parameter("sinb", [128, NSB], f32, isOutput=False)
    bias12 = nc.declare_dram_parameter("bias12", [128, 2 * MC], f32, isOutput=False)
    out = nc.declare_dram_parameter("out", [N - 1, BPC * N], f32, isOutput=True)

    with tile.TileContext(nc) as tc:
        with (
            tc.tile_pool(name="const", bufs=1) as cpool,
            tc.tile_pool(name="xw", bufs=1) as xwpool,
            tc.tile_pool(name="y", bufs=1) as ypool,
            tc.tile_pool(name="feat", bufs=1) as fpool,
            tc.tile_pool(name="stage", bufs=4) as stpool,
        ):
            # ---- input DMAs: one DMA covers x + side-0 weights ----
            H = MC * KC * 128
            b0sb = xwpool.tile([128, KC * TOK + H], bf16, tag="b0sb")
            nc.sync.dma_start(b0sb[:, :], big0[:, :])
            xsb = b0sb
            b1sb = xwpool.tile([128, H], bf16, tag="b1sb")
            nc.gpsimd.dma_start(b1sb[:, :], big1[:, :])
            NSB = len(STRUCT) * 2 * MC
            snb = cpool.tile([128, NSB], f32, tag="snb")
            nc.scalar.dma_start(snb[:, :], sinb[:, :])
            b12 = cpool.tile([128, 2 * MC], f32, tag="b12")
            nc.scalar.dma_start(b12[:, :], bias12[:, :])
            wco = cpool.tile([128, NWC], f32, tag="wco")
            nc.scalar.dma_start(wco[:, :], wcoef[:, :])
            NWD = sum(1 for bb, L in STRUCT for m in range(L - 1)) * MC * 2
            wdu = cpool.tile([128, NWD], bf16, tag="wdu")
            nc.scalar.dma_start(wdu[:, :], wdup[:, :])
            boutf = cpool.tile([128, 1], f32, tag="boutf")
            nc.scalar.dma_start(boutf[:, :], boutp[:, :])
            halfpi = cpool.tile([128, 1], f32, tag="halfpi")
            nc.gpsimd.memset(halfpi[:, :], PI / 2)

            def wslice(side, c, k):
                o = (c * KC + k) * 128
                if side == 0:
                    return b0sb[:, KC * TOK + o:KC * TOK + o + 128]
                return b1sb[:, o:o + 128]

            warm = cpool.tile([128, 128], bf16, tag="warm")
            nc.gpsimd.memset(warm[:, :], 0.25)
            with (
                tc.tile_pool(name="psF", bufs=1, space=bass.MemorySpace.PSUM) as psF,
                tc.tile_pool(name="psS", bufs=4, space=bass.MemorySpace.PSUM) as psS,
                tc.tile_pool(name="psX", bufs=2, space=bass.MemorySpace.PSUM) as psX,
                tc.tile_pool(name="psO", bufs=1, space=bass.MemorySpace.PSUM) as psO,
            ):
                fps = psF.tile([128, 128], f32, tag="psF")

                def filler(n, dep=None):
                    rhs = warm[:, :] if dep is None else dep[:, 0:128]
                    for _ in range(n):
                        nc.tensor.matmul(fps[:, :], warm[:, :], rhs,
                                         start=True, stop=True)
                # ---- x matmuls. side0 psums stay resident for direct Sin;
                # side1 staged to SBUF via ACT Identity (+bias) ----
                y1 = ypool.tile([128, MC * TOK], f32, tag="y1", name="y1")
                ps0 = []
                for c in range(MC):
                    ps = psS.tile([128, TOK], f32, tag="psS", name=f"ps0_{c}")
                    for k in range(KC):
                        nc.tensor.matmul(ps[:, :], wslice(0, c, k),
                                         xsb[:, k * TOK:(k + 1) * TOK],
                                         start=(k == 0), stop=(k == KC - 1))
                    ps0.append(ps)
                for c in range(MC):
                    ps = psX.tile([128, TOK], f32, tag="psX", name=f"psx1_{c}")
                    for k in range(KC):
                        nc.tensor.matmul(ps[:, :], wslice(1, c, k),
                                         xsb[:, k * TOK:(k + 1) * TOK],
                                         start=(k == 0), stop=(k == KC - 1))
                    nc.vector.tensor_scalar(
                        y1[:, c * TOK:(c + 1) * TOK], ps[:, :],
                        b12[:, MC + c:MC + c + 1], None, ALU.add)

                pout_t = psO.tile([N - 1, BPC * N], f32, tag="psO", name="pout")
                pouts = [pout_t[:, b * N:(b + 1) * N] for b in range(BPC)]
                zro = cpool.tile([128, BPC * N], bf16, tag="zro")
                nc.gpsimd.memset(zro[:, :], 0.0)
                nc.tensor.matmul(pout_t[:, :], warm[:, 0:N - 1], zro[:, :],
                                 start=True, stop=False, skip_group_check=True)
                FW = MC * TOK

                st = {}       # (li, side, m) -> (s_tile, c_tile)
                sq = {}       # (li, side, m) -> sigma^2 tile
                mm_first = True
                dmaq = [nc.sync, nc.scalar, nc.sync, nc.scalar]

                # wcoef column offsets per (line, m)
                wc_off = []
                o = 0
                for li, (bb, L) in enumerate(STRUCT):
                    offs = []
                    for m in range(L):
                        offs.append(o)
                        o += 4 if m == 0 else 8
                    wc_off.append(offs)
                # wdup column offsets per (line, m<=L-2): MC*2 bf16 cols each
                wd_off = []
                o = 0
                for li, (bb, L) in enumerate(STRUCT):
                    offs = []
                    for m in range(L - 1):
                        offs.append(o)
                        o += MC * 2
                    wd_off.append(offs)

                def make_cw_tt(li, m, side, c_tile):
                    # cw = c_m * (w_a*coef) via TT-mult against dup-pair
                    # broadcast AP (keeps DVE 2x mode)
                    cw = fpool.tile([128, FW], bf16, tag=f"cw_{li}_{side}_p{m % 2}",
                                    name=f"cw{li}{side}{m}")
                    wap = wdu[:, :]
                    in1 = bass.AP(wap.tensor, wap.offset + wd_off[li][m],
                                  [[wap.ap[0][0], 128], [2, MC], [0, TOK // 2],
                                   [1, 2]])
                    cap = c_tile[:, :]
                    in0 = bass.AP(cap.tensor, cap.offset,
                                  [[cap.ap[0][0], 128], [TOK, MC], [2, TOK // 2],
                                   [1, 2]])
                    oap = cw[:, :]
                    o0 = bass.AP(oap.tensor, oap.offset,
                                 [[oap.ap[0][0], 128], [TOK, MC], [2, TOK // 2],
                                  [1, 2]])
                    nc.vector.tensor_tensor(o0, in0, in1, ALU.mult)
                    return cw

                def make_cw_leaf(li, m, side, sq_tile):
                    # leaf: cw = sigma^2*(-2 w coef/lam^2) + w coef (dual ts)
                    # as per-chunk tiles so final mms start on chunk 0
                    cws = []
                    for c in range(MC):
                        cwc = fpool.tile([128, TOK], bf16,
                                         tag=f"cwl_{li}_{side}_{c}",
                                         name=f"cwl{li}{side}{m}{c}")
                        col = wc_off[li][m] + 2 * c
                        nc.vector.tensor_scalar(
                            cwc[:, :],
                            sq_tile[:, c * TOK:(c + 1) * TOK],
                            wco[:, col:col + 1], wco[:, col + 1:col + 2],
                            ALU.mult, ALU.add)
                        cws.append(cwc)
                    return cws

                def emit_mms(li, m, cws, last):
                    # A: psum[i,j] += sum_a c2w[a,i]*s1[a,j]
                    # B: psum[i,j] += sum_a s2[a,i]*c1w[a,j]
                    nonlocal mm_first
                    s1_t = st[(li, 0, m)][0]
                    s2_t = st[(li, 1, m)][0]
                    c1w_t, c2w_t = cws

                    def cw_sl(t, c, lo, hi):
                        if isinstance(t, list):
                            return t[c][:, lo:hi]
                        return t[:, c * TOK + lo:c * TOK + hi]
                    if not last:
                        for c in range(MC):
                            for b in range(BPC):
                                t0 = c * TOK + b * N
                                nc.tensor.matmul(
                                    pouts[b],
                                    cw_sl(c2w_t, c, b * N + 1, b * N + N),
                                    s1_t[:, t0:t0 + N],
                                    start=False, stop=False,
                                    skip_group_check=True)
                                nc.tensor.matmul(
                                    pouts[b],
                                    s2_t[:, t0 + 1:t0 + N],
                                    cw_sl(c1w_t, c, b * N, b * N + N),
                                    start=False, stop=False,
                                    skip_group_check=True)
                        mm_first = False
                        return
                    # final pair: batch-major mms, stage into one wide tile,
                    # two big (>=4KB/partition-row) DMAs
                    stg = stpool.tile([N - 1, BPC * N], f32, tag="stg")
                    for b in range(BPC):
                        for c in range(MC):
                            t0 = c * TOK + b * N
                            nc.tensor.matmul(
                                pouts[b],
                                cw_sl(c2w_t, c, b * N + 1, b * N + N),
                                s1_t[:, t0:t0 + N],
                                start=False, stop=False,
                                skip_group_check=True)
                            nc.tensor.matmul(
                                pouts[b],
                                s2_t[:, t0 + 1:t0 + N],
                                cw_sl(c1w_t, c, b * N, b * N + N),
                                start=False,
                                stop=(c == MC - 1),
                                skip_group_check=True)
                        nc.scalar.activation(stg[:, b * N:(b + 1) * N],
                                             pouts[b], AF.Identity,
                                             bias=boutf[0:N - 1, 0:1])
                        if b == 1:
                            nc.sync.dma_start(out[:, 0:2 * N], stg[:, 0:2 * N])
                        elif b == BPC - 1:
                            nc.scalar.dma_start(out[:, 2 * N:], stg[:, 2 * N:])

                # ---- bases: per line, side0 direct from PSUM then side1
                # from y1, so ACT starts as soon as side-0 psums land ----
                for li, (bb, L) in enumerate(STRUCT):
                    for ph in range(2):
                        f0 = fpool.tile([128, FW], bf16,
                                        tag=("s" if ph == 0 else "c") + f"_{li}_0_p0",
                                        name=("s" if ph == 0 else "c") + f"{li}00")
                        for c in range(MC):
                            col = (li * 2 + ph) * MC + c
                            nc.scalar.activation(f0[:, c * TOK:(c + 1) * TOK],
                                                 ps0[c][:, :], AF.Sin,
                                                 bias=snb[:, col:col + 1],
                                                 scale=float(bb))
                        if ph == 0:
                            s0_0 = f0
                        else:
                            st[(li, 0, 0)] = (s0_0, f0)
                    s0 = fpool.tile([128, FW], bf16, tag=f"s_{li}_1_p0",
                                    name=f"s{li}10")
                    nc.scalar.activation(s0[:, :], y1[:, :], AF.Sin,
                                         bias=0.0, scale=float(bb))
                    c0 = fpool.tile([128, FW], bf16, tag=f"c_{li}_1_p0",
                                    name=f"c{li}10")
                    nc.scalar.activation(c0[:, :], y1[:, :], AF.Sin,
                                         bias=halfpi[:, 0:1], scale=float(bb))
                    st[(li, 1, 0)] = (s0, c0)
                    for side in range(2):
                        t = fpool.tile([128, FW], bf16, tag=f"sq_{li}_{side}",
                                       name=f"sq{li}{side}0")
                        nc.scalar.activation(t[:, :], st[(li, side, 0)][0][:, :],
                                             AF.Square)
                        sq[(li, side, 0)] = t
                    cws = [make_cw_tt(li, 0, side, st[(li, side, 0)][1])
                           for side in range(2)]
                    emit_mms(li, 0, cws, last=False)
                    filler(2, dep=st[(li, 0, 0)][0])


                # ---- staggered level schedule ----
                steps = []
                Lmax = max(L for _, L in STRUCT)
                for m in range(1, Lmax):
                    for li, (bb, L) in enumerate(STRUCT):
                        if m < L:
                            steps.append((li, m))
                for si, (li, m) in enumerate(steps):
                    bb, L = STRUCT[li]
                    lam_prev = 0.5 ** (m - 1)
                    is_last = si == len(steps) - 1
                    cms = [None, None]
                    if m <= L - 2:
                        for side in range(2):
                            cm = fpool.tile([128, FW], bf16,
                                            tag=f"c_{li}_{side}_p{m % 2}",
                                            name=f"c{li}{side}{m}")
                            nc.vector.tensor_scalar(
                                cm[:, :], sq[(li, side, m - 1)][:, :],
                                float(-2.0 / lam_prev ** 2), 1.0,
                                ALU.mult, ALU.add)
                            cms[side] = cm

                    def make_cw_for(side):
                        if m <= L - 2:
                            return make_cw_tt(li, m, side, cms[side])
                        return make_cw_leaf(li, m, side, sq[(li, side, m - 1)])

                    def make_s(side):
                        sp, cp = st[(li, side, m - 1)]
                        sm = fpool.tile([128, FW], bf16,
                                        tag=f"s_{li}_{side}_p{m % 2}",
                                        name=f"s{li}{side}{m}")
                        nc.vector.tensor_tensor(sm[:, :], sp[:, :], cp[:, :],
                                                ALU.mult)
                        return sm

                    if m <= L - 2:
                        cw1 = make_cw_for(1)
                        s1m = make_s(0)
                        cw0 = make_cw_for(0)
                        s2m = make_s(1)
                    else:
                        s1m = make_s(0)
                        s2m = make_s(1)
                        filler(6, dep=s1m)
                        cw1 = []
                        cw0 = []
                        for c in range(MC):
                            for side, lst in ((1, cw1), (0, cw0)):
                                cwc = fpool.tile([128, TOK], bf16,
                                                 tag=f"cwl_{li}_{side}_{c}",
                                                 name=f"cwl{li}{side}{m}{c}")
                                col = wc_off[li][m] + 2 * c
                                nc.vector.tensor_scalar(
                                    cwc[:, :],
                                    sq[(li, side, m - 1)][:, c * TOK:(c + 1) * TOK],
                                    wco[:, col:col + 1], wco[:, col + 1:col + 2],
                                    ALU.mult, ALU.add)
                                lst.append(cwc)
                    st[(li, 0, m)] = (s1m, cms[0])
                    st[(li, 1, m)] = (s2m, cms[1])
                    if m <= L - 2:
                        for side, smt in ((0, s1m), (1, s2m)):
                            t = fpool.tile([128, FW], bf16, tag=f"sq_{li}_{side}",
                                           name=f"sq{li}{side}{m}")
                            nc.scalar.activation(t[:, :], smt[:, :], AF.Square)
                            sq[(li, side, m)] = t
                    emit_mms(li, m, [cw0, cw1], last=is_last)
                    if not is_last:
                        filler(2, dep=st[(li, 0, m - 1)][0])

    nc.finalize()
    return nc


def _get_nc():
    if "nc" not in _CACHE:
        _CACHE["nc"] = _build_nc()
    return _CACHE["nc"]


def _prep_in_maps(x, W1, b1, W2, b2, Wout, bout):
    import ml_dtypes
    f = np.float32
    bfd = ml_dtypes.bfloat16
    x = np.asarray(x, f)
    W1 = np.asarray(W1, f)
    W2 = np.asarray(W2, f)
    Wo = np.asarray(Wout, f)

    # weights: wpk[d', ((side*MC+c)*KC+k)*128 + j] = Wside[c*128+j, k*128+d']
    def pack_w(W):
        # [MC, 128j, KC, 128d'] -> transpose to [128d', MC, KC, 128j]
        t = W.reshape(MC, 128, KC, 128).transpose(3, 0, 2, 1)
        return np.ascontiguousarray(t.reshape(128, MC * KC * 128)).astype(f)

    w1p = pack_w(W1)
    w2p = pack_w(W2)
    bias12 = np.ascontiguousarray(
        np.concatenate([np.asarray(b1, f).reshape(MC, 128).T,
                        np.asarray(b2, f).reshape(MC, 128).T], axis=1))
    NSB = len(STRUCT) * 2 * MC
    snb = np.zeros((128, NSB), f)
    b1c = np.asarray(b1, f).reshape(MC, 128).T
    for li, (bb, L) in enumerate(STRUCT):
        for ph in range(2):
            for c in range(MC):
                snb[:, (li * 2 + ph) * MC + c] = bb * b1c[:, c] + (0.0 if ph == 0 else PI / 2)

    # wcoef table
    NWC = sum((4 if m == 0 else 8) for bb, L in STRUCT for m in range(L))
    wco = np.zeros((128, NWC), f)
    o = 0
    ki = 0
    for bb, L in STRUCT:
        for m in range(L):
            lam = 0.5 ** m
            coef = _BK[ki + m] / lam
            if m == 0:
                for c in range(MC):
                    wco[:, o + c] = Wo[c * 128:(c + 1) * 128] * coef
                o += 4
            else:
                lam_prev = 0.5 ** (m - 1)
                for c in range(MC):
                    wchunk = Wo[c * 128:(c + 1) * 128]
                    wco[:, o + 2 * c] = wchunk * (-2.0 * coef / lam_prev ** 2)
                    wco[:, o + 2 * c + 1] = wchunk * coef
                o += 8
        ki += L
    bov = np.full((128, 1), np.asarray(bout, f).reshape(()), f)

    # duplicated-pair bf16 table for inner cw TT: per (line, m<=L-2, chunk)
    NWD = sum(1 for bb, L in STRUCT for m in range(L - 1)) * MC * 2
    wdu = np.zeros((128, NWD), f)
    o = 0
    ki = 0
    for bb, L in STRUCT:
        for m in range(L - 1):
            coef = _BK[ki + m] / (0.5 ** m)
            for c in range(MC):
                v = Wo[c * 128:(c + 1) * 128] * coef
                wdu[:, o + 2 * c] = v
                wdu[:, o + 2 * c + 1] = v
            o += MC * 2
        ki += L
    wdu = wdu.astype(bfd)

    in_maps = []
    for ci in range(NCORES):
        xs = x[ci * BPC:(ci + 1) * BPC]          # [BPC, N, D]
        xt = xs.transpose(2, 0, 1).reshape(D, TOK)   # [D, TOK]
        # -> [128, KC*TOK]: xTi[d', k*TOK + t] = xt[k*128+d', t]
        xTi = np.ascontiguousarray(
            xt.reshape(KC, 128, TOK).transpose(1, 0, 2).reshape(128, KC * TOK))
        in_maps.append({
            "big0": np.ascontiguousarray(np.concatenate([xTi.astype(np.float32),
                                                         w1p], axis=1)).astype(bfd),
            "big1": w2p.astype(bfd), "bias12": bias12, "sinb": snb,
            "wcoef": wco, "bout": bov, "wdup": wdu,
        })
    return in_maps


def _run(x, W1, b1, W2, b2, Wout, bout, trace=False):
    from concourse.bass_utils import run_bass_kernel_spmd

    nc = _get_nc()
    in_maps = _prep_in_maps(x, W1, b1, W2, b2, Wout, bout)
    res = run_bass_kernel_spmd(nc, in_maps, core_ids=list(range(NCORES)),
                               trace=trace)
    outs = [np.asarray(res.results[ci]["out"]).reshape(N - 1, BPC, N)
            .transpose(1, 0, 2) for ci in range(NCORES)]
    full = np.concatenate(outs, axis=0).astype(np.float32)
    return full, res


def kernel(x, W1, b1, W2, b2, Wout, bout):
    full, _ = _run(x, W1, b1, W2, b2, Wout, bout, trace=False)
    return full
